# revision 62
# baseline (speedup 1.0000x reference)
"""Trainium2 Bass kernel for nn_Attention_50989851738305.

The reference module applies jnp.tril(scores, k=-999999) which zeroes the
entire score matrix (S=2048 << 999999), so softmax is uniform 1/S and the
attention output reduces exactly to

    out[b, s, :] = (mean_s' hidden[b, s', :]) @ Wv.T @ Wo.T   (constant in s)

Wq/Wk are mathematically irrelevant. Per core (sequence sharded 8x):

  - Wv/Wo are folded ON DEVICE once per kernel invocation into
    M = Wv.T @ Wo.T (4MB, SBUF-resident), so each iteration needs ONE
    cross-core exchange (8KB of full-sequence partial column sums).
  - The bulk 2MB hidden load and 2MB output store are each split across
    BOTH HWDGE rings (sync + scalar).
  - r[b] = hbar[b] @ M is computed fused with the 128-partition broadcast
    (lhsT column p is hbar[p//64, :], so PSUM row p = r[b(p)]), matching
    the out[(p c)] store pattern where output row 4p+c belongs to batch
    p//64 for every c.

EXCHANGE modes for the 8KB cross-core partial-sum exchange (default "agb",
the shipped configuration; the others are kept for A/B probing):
  "agb":    PRODUCTION: bulk data (hidden load, row sums, M, output store)
            in bf16 -- halves the dominant HBM traffic; partial column
            sums, the exchange, and all matmul accumulation stay f32, so
            measured rel err is ~4.6e-3 against the f32 reference (the
            harness gate is 2e-2). Exchange = firmware AllGather of the
            transposed partials (f-major [16,128] blocks, so the gathered
            [128,128] fetch is contiguous) + one PE mask-matmul that sums
            the 8 rank blocks and lands hbT in the [q, (dc b)] layout the
            tail wants. The 2MB hidden slice loads as two half-tiles
            whose folds feed accumulating PE matmuls (start/stop pairs
            adjacent per PSUM region), so the exchange kickoff only waits
            on half the load. Measured (interleaved paired two-K slope
            diff, distinct-slice bodies): ~7.9us/body in clean windows
            (~13us under heavy shared-machine load) vs ~24.7us for the
            f32 AllReduce baseline and ~8.4us for the f32 exchange-free
            floor; rel err 3.0e-3.
  "ncfw":   f32 + firmware AllReduce (the previous baseline).
  "ag"/"ag2"/"agalt"/"ags"/"agp": f32 AllGather variants (probing).
  "none":   no exchange (numerically wrong, DMA/compute floor probe only).
  "remote"/"remote2"/"remote3": XOR-slot all-gather over
            remote_dma_broadcast (probing; tile_critical overhead makes
            all of them slower than the firmware path here).
"""
import numpy as np

import concourse.bass as bass  # noqa: F401  (bass registers engine types)
import concourse.tile as tile
from concourse import bacc, mybir
from concourse.bass_utils import run_bass_kernel_spmd

B = 2
S = 2048
D = 1024
N_CORES = 8
S_LOC = S // N_CORES          # 256 sequence rows per core (per batch)
ROWS = B * S_LOC              # 512 rows of the local hidden slice
SCALE = 1.0 / S               # uniform attention weight (exact power of two)
F32 = mybir.dt.float32
BF16 = mybir.dt.bfloat16
DC = D // 128                 # 8 chunks of the model dim
GROUP = [list(range(N_CORES))]

EXCHANGE = "agb"              # see docstring
BF_MODES = {"agb", "agbx", "noneb"}   # bulk data in bf16
_BUILT = {}
_ITER = [0]                   # emission counter for cumulative sem targets


def _emit_const(nc, tc, pool, psum_m, wv_d, wot_d, exchange, aux_d):
    """Once per kernel: constants + fold M = Wv.T @ Wo.T (resident, 4MB)."""
    bdt = BF16 if exchange in BF_MODES else F32
    # masked ones: col b has 1/S on partitions where b(p) = p//64 == b
    # (1/2048 is a power of two: exact in bf16)
    ones2m = pool.tile([128, 2], bdt, tag="ones2m", bufs=1)
    nc.vector.memset(ones2m[:], 0.0)
    nc.vector.memset(ones2m[0:64, 0:1], SCALE)
    nc.vector.memset(ones2m[64:128, 1:2], SCALE)

    mask16 = ident = None
    if exchange.startswith("ag"):
        # host-prepared constants (BIR forbids per-partition memset builds):
        # aux[:, 0:16]  = mask16[k*16+f, f'] = 1.0 iff f == f'
        # aux[:, 16:144] = identity for the pT -> pTt PE transpose
        aux = pool.tile([128, 144], F32, tag="aux", bufs=1)
        nc.sync.dma_start(aux[:], aux_d.ap())
        mask16 = aux[:, 0:16]
        ident = aux[:, 16:144]
        if exchange == "agbx":
            # bf16 copy of the reduce mask (PE operands must match dtype)
            mask16b = pool.tile([128, 16], BF16, tag="mask16b", bufs=1)
            nc.vector.tensor_copy(mask16b[:], mask16)
            mask16 = mask16b

    wv_sb = pool.tile([128, DC, D], F32, tag="wv", bufs=1)
    nc.sync.dma_start(wv_sb[:], wv_d.ap().rearrange("(c p) d -> p c d", p=128))
    wot_sb = pool.tile([128, DC, D], F32, tag="wot", bufs=1)
    nc.scalar.dma_start(wot_sb[:], wot_d.ap().rearrange("(c p) d -> p c d", p=128))

    # M[d, dout] = sum_j Wv[j, d] * WoT[j, dout], laid out [p, dc, dout]
    m_sb = pool.tile([128, DC, D], bdt, tag="m", bufs=1)
    for dc in range(DC):
        m_ps = psum_m.tile([128, D], F32, tag="mps", bufs=1)
        for half in range(2):
            for jc in range(DC):
                nc.tensor.matmul(
                    m_ps[:, half * 512:(half + 1) * 512],
                    wv_sb[:, jc, dc * 128:(dc + 1) * 128],
                    wot_sb[:, jc, half * 512:(half + 1) * 512],
                    start=(jc == 0),
                    stop=(jc == DC - 1),
                )
        nc.vector.tensor_copy(m_sb[:, dc, :], m_ps[:])
    return ones2m, m_sb, mask16, ident


def _emit_exchange_ncfw(nc, tc, pool, dram, pT_sb):
    """hbT = AllReduce(pT) over ncfw."""
    cc_in = dram.tile([128, 2 * DC], F32, tag="cci")
    cc_out = dram.tile([128, 2 * DC], F32, tag="cco", addr_space="Shared")
    nc.gpsimd.dma_start(cc_in[:], pT_sb[:])
    nc.gpsimd.collective_compute(
        "AllReduce", mybir.AluOpType.add, replica_groups=GROUP,
        ins=[cc_in.opt()], outs=[cc_out.opt()],
    )
    hbT = pool.tile([128, 2 * DC], F32, tag="hbT")
    nc.gpsimd.dma_start(hbT[:], cc_out[:])
    return hbT


def _emit_exchange_ag(nc, tc, pool, psum_x, dram, pTt_sb, mask16, hwdge,
                      cc_eng=None, xdt=F32):
    """hbT = sum of AllGather'd transposed partials, via one PE matmul.

    pTt_sb: [16, 128] f-major transposed partials (f = dc*2+b, q = d%128).
    AllGather stacks the 8 ranks' 8KB blocks -> cc_out [128, 128] where
    row k*16+f holds rank k's f-row. One matmul with lhsT = gathered tile
    and rhs = mask16 sums over k and transposes back to [q, f].

    hwdge: issue the HBM bounce copies on the sync/scalar queues so the
    gpsimd queue only carries the blocking collective itself.
    """
    cc_in = dram.tile([16, 128], xdt, tag="cci")
    cc_out = dram.tile([128, 128], xdt, tag="cco", addr_space="Shared")
    (nc.sync if hwdge else nc.gpsimd).dma_start(cc_in[:], pTt_sb[:])
    (cc_eng or nc.gpsimd).collective_compute(
        "AllGather", mybir.AluOpType.bypass, replica_groups=GROUP,
        ins=[cc_in.opt()], outs=[cc_out.opt()],
    )
    g_sb = pool.tile([128, 128], xdt, tag="g")
    (nc.scalar if hwdge else nc.gpsimd).dma_start(g_sb[:], cc_out[:])
    hbT_ps = psum_x.tile([128, 2 * DC], F32, tag="hbTps", bufs=1)
    nc.tensor.matmul(hbT_ps[:], g_sb[:], mask16[:], start=True, stop=True)
    hbT = pool.tile([128, 2 * DC], F32, tag="hbT")
    nc.vector.tensor_copy(hbT[:], hbT_ps[:])
    return hbT


def _emit_exchange_remote3(nc, tc, pool, pT_sb, rit):
    """XOR-slot all-gather with ALL exchange ops inside one Pool-only
    tile_critical per body.

    Why this shape:
      - The Tile scheduling sim cannot model remote semaphore increments,
        so manual rs-waits deadlock it OUTSIDE criticals; critical contents
        are exempt.
      - tile_critical drains only the engines used INSIDE it; with
        no_gpsimd_drain=True the Pool drain is a NOP, so DVE/PE/ACT/SP
        never stall and bodies pipeline freely.
      - Criticals chain serially, which is exactly the Pool-serial order
        the exchange needs anyway.

    Slot safety (g bufs=4): sends of round r are ordered after the local
    reduce of r-1 via the consumed gate; a peer's write of round r+4 into
    my slot r%4 requires its reduce r+3, which requires my send r+3, which
    requires my reduce r+2 > r.

    Arrival counting uses one semaphore per slot (rs[w], w = rit%4):
    threshold 14*(rit//4+1). A peer would have to run 4 whole rounds ahead
    to overcount a slot sem, which the consumed gate makes structurally
    impossible -- so the wait implies all 7 peers' round-rit data landed.
    """
    g = pool.tile([128, 8, 2 * DC], F32, tag="g", bufs=4)
    nc.vector.tensor_copy(g[:, 0, :], pT_sb[:])
    w = rit % 4
    nsends = getattr(nc, "_nsends", 7)
    with tc.tile_critical(no_gpsimd_drain=True):
        if rit >= 1:
            nc.gpsimd.wait_ge(nc._consumed, rit)
        for k in range(1, 1 + nsends):
            rdests = [None] * 8
            rdests[k] = (0, k)
            nc.gpsimd.remote_dma_broadcast(
                g[:, k, :], g[:, 0, :],
                remote_sem=nc._rs[w], local_sem=nc._ls1,
                rdests=rdests,
            ).then_inc(nc._prep1, 1)
        nc.gpsimd.wait_ge(nc._prep1, nsends * (rit + 1))
        nc.gpsimd.trigger_dma(nsends)
        nc.gpsimd.wait_ge(nc._rs[w], 2 * nsends * (rit // 4 + 1))
    hbT = pool.tile([128, 2 * DC], F32, tag="hbT")
    nc.vector.tensor_reduce(
        hbT[:], g[:].rearrange("p k f -> p f k"),
        mybir.AxisListType.X, mybir.AluOpType.add,
    )
    # consumed inc rides a nop: the reduce already carries Tile's own sync
    # updates and the TR encoding has no free slot ("Too many sync update
    # commands"); DVE is in-order so nop-completion == reduce done.
    nc.vector.nop(nofuse=True, hint="consumed_inc").then_inc(nc._consumed, 1)
    return hbT


def _emit_exchange_remote2(nc, tc, pool, pT_sb, rit):
    """XOR-slot all-gather without tile_critical (those drain every engine
    at exit -- two per body made the v1 path ~85us/body).

    Slot window: g bufs=4; sends of round r are gated on consumed >= r
    (reduce r-1 done locally). Safety chain: peer's write of round r+4
    into my slot requires peer reduce r+2, which requires my send r+2,
    which requires my reduce r+1 > my reduce r -- so slot r%4 is long
    consumed before it is overwritten. In steady state every wait here is
    pre-satisfied (exchange latency ~3us << round period), so no engine
    stalls."""
    g = pool.tile([128, 8, 2 * DC], F32, tag="g", bufs=4)
    nc.vector.tensor_copy(g[:, 0, :], pT_sb[:])
    if rit >= 1:
        nc.gpsimd.wait_ge(nc._consumed, rit)
    for k in range(1, 8):
        rdests = [None] * 8
        rdests[k] = (0, k)
        nc.gpsimd.remote_dma_broadcast(
            g[:, k, :], pT_sb[:],
            remote_sem=nc._rs1, local_sem=nc._ls1,
            rdests=rdests,
        ).then_inc(nc._prep1, 1)
    nc.gpsimd.wait_ge(nc._prep1, 7 * (rit + 1))
    nc.gpsimd.trigger_dma(7)
    hbT = pool.tile([128, 2 * DC], F32, tag="hbT")
    nc.vector.wait_ge(nc._rs1, 14 * (rit + 1))
    nc.vector.tensor_reduce(
        hbT[:], g[:].rearrange("p k f -> p f k"),
        mybir.AxisListType.X, mybir.AluOpType.add,
    ).then_inc(nc._consumed, 1)
    return hbT


def _emit_exchange_remote(nc, tc, pool, pT_sb, rit):
    """hbT = sum over the 8 cores' pT via XOR-slot all-gather + DVE reduce.

    rit is the 0-based REMOTE iteration index (iteration 0 of the program
    always goes through ncfw, whose firmware rendezvous guarantees every
    core is executing -- with per-execution-zeroed semaphores -- before the
    first remote send fires)."""
    g = pool.tile([128, 8, 2 * DC], F32, tag="g", bufs=4)
    nc.vector.tensor_copy(g[:, 0, :], pT_sb[:])
    with tc.tile_critical():
        if rit >= 1:
            # send(rit) >= consume(rit-1) bounds inter-core skew
            nc.gpsimd.wait_ge(nc._consumed, rit)
        for k in range(1, 8):
            rdests = [None] * 8
            rdests[k] = (0, k)
            nc.gpsimd.remote_dma_broadcast(
                g[:, k, :], pT_sb[:],
                remote_sem=nc._rs1, local_sem=nc._ls1,
                rdests=rdests,
            ).then_inc(nc._prep1, 1)
        nc.gpsimd.wait_ge(nc._prep1, 7 * (rit + 1))
        nc.gpsimd.trigger_dma(7)
    hbT = pool.tile([128, 2 * DC], F32, tag="hbT")
    with tc.tile_critical():
        nc.vector.wait_ge(nc._rs1, 14 * (rit + 1))
        nc.vector.tensor_reduce(
            hbT[:], g[:].rearrange("p k f -> p f k"),
            mybir.AxisListType.X, mybir.AluOpType.add,
        ).then_inc(nc._consumed, 1)
    return hbT


def _emit_body_a(nc, tc, pools, const, h_d, big_h):
    """Pipelined-AG stage A: load + partial sums + AllGather doorbell.

    Returns the ctx stage B needs. The gpsimd queue only carries the 8KB
    cc_in store and the PTC doorbell here -- the PTC releases the sequencer
    at doorbell time (collective runs on the TOPSP cores), so consecutive
    bodies' AllGathers overlap once the fetch is deferred to stage B.
    """
    pool, psum1, psum2, dram = pools
    ones2m, m_sb, mask16, ident = const
    it = _ITER[0]
    _ITER[0] += 1

    h_sb = pool.tile([128, 4, D], F32, tag="h", bufs=4)
    if big_h:
        hap = h_d.ap().rearrange("(i p c) d -> i p c d", i=16, p=128)[it % 16]
    else:
        hap = h_d.ap().rearrange("(p c) d -> p c d", p=128)
    nc.sync.dma_start(h_sb[:, 0:1, :], hap[:, 0:1, :])
    nc.scalar.dma_start(h_sb[:, 1:2, :], hap[:, 1:2, :])
    nc.sync.dma_start(h_sb[:, 2:3, :], hap[:, 2:3, :])
    nc.scalar.dma_start(h_sb[:, 3:4, :], hap[:, 3:4, :])

    hsum = pool.tile([128, D], F32, tag="hsum")
    nc.vector.tensor_tensor(hsum[:], h_sb[:, 0, :], h_sb[:, 1, :],
                            mybir.AluOpType.add)
    nc.vector.tensor_tensor(hsum[:], hsum[:], h_sb[:, 2, :],
                            mybir.AluOpType.add)
    nc.vector.tensor_tensor(hsum[:], hsum[:], h_sb[:, 3, :],
                            mybir.AluOpType.add)

    pT_ps = psum1.tile([128, 2 * DC], F32, tag="pT")
    for dc in range(DC):
        nc.tensor.matmul(
            pT_ps[:, dc * 2:dc * 2 + 2],
            hsum[:, dc * 128:(dc + 1) * 128],
            ones2m[:],
            start=True,
            stop=True,
        )
    pT_sb = pool.tile([128, 2 * DC], F32, tag="pTs")
    nc.vector.tensor_copy(pT_sb[:], pT_ps[:])

    pTt_ps = psum1.tile([2 * DC, 128], F32, tag="pTt", bufs=1)
    nc.tensor.matmul(pTt_ps[:], pT_sb[:], ident[:], start=True, stop=True)
    pTt_sb = pool.tile([2 * DC, 128], F32, tag="pTts")
    nc.vector.tensor_copy(pTt_sb[:], pTt_ps[:])

    cc_in = dram.tile([16, 128], F32, tag="cci")
    cc_out = dram.tile([128, 128], F32, tag="cco", addr_space="Shared")
    nc.gpsimd.dma_start(cc_in[:], pTt_sb[:])
    nc.gpsimd.collective_compute(
        "AllGather", mybir.AluOpType.bypass, replica_groups=GROUP,
        ins=[cc_in.opt()], outs=[cc_out.opt()],
    )
    return cc_out


def _emit_body_b(nc, tc, pools, const, out_d, cc_out):
    """Pipelined-AG stage B: fetch gathered partials + compute + store."""
    pool, psum1, psum2, dram = pools
    ones2m, m_sb, mask16, ident = const

    g_sb = pool.tile([128, 128], F32, tag="g")
    nc.gpsimd.dma_start(g_sb[:], cc_out[:])
    hbT_ps = psum1.tile([128, 2 * DC], F32, tag="hbTps", bufs=1)
    nc.tensor.matmul(hbT_ps[:], g_sb[:], mask16[:], start=True, stop=True)
    hbT = pool.tile([128, 2 * DC], F32, tag="hbT")
    nc.vector.tensor_copy(hbT[:], hbT_ps[:])

    selb = pool.tile([128, DC, 2, 64], F32, tag="selb")
    nc.vector.tensor_copy(
        selb[:],
        hbT[:].rearrange("p (dc b) -> p dc b", b=2)
              .unsqueeze(3).broadcast_to([128, DC, 2, 64]),
    )

    bc_ps = psum2.tile([128, D], F32, tag="bc")
    for half in range(2):
        for dc in range(DC):
            nc.tensor.matmul(
                bc_ps[:, half * 512:(half + 1) * 512],
                selb[:, dc, :, :].rearrange("p b r -> p (b r)"),
                m_sb[:, dc, half * 512:(half + 1) * 512],
                start=(dc == 0),
                stop=(dc == DC - 1),
            )
    bc_sb = pool.tile([128, D], bdt, tag="bcs", bufs=3)
    nc.scalar.copy(bc_sb[:], bc_ps[:])

    oap = out_d.ap().rearrange("(p c) d -> p c d", p=128)
    for c in range(4):
        eng = nc.sync if c % 2 == 0 else nc.scalar
        eng.dma_start(oap[:, c:c + 1, :],
                      bc_sb[:].unsqueeze(1).broadcast_to([128, 1, D]))


def _emit_body(nc, tc, pools, const, h_d, out_d, exchange, big_h):
    pool, psum1, psum2, dram = pools
    ones2m, m_sb, mask16, ident = const
    it = _ITER[0]
    bdt = BF16 if exchange in BF_MODES else F32

    if exchange in ("agonly", "agburst"):
        # exchange-chain-only probes: "agonly" = serial store+AG+fetch per
        # body (latency), "agburst" = store+doorbell only, 4-deep window
        # (ncfw AllGather throughput)
        _ITER[0] += 1
        pTt_sb = pool.tile([16, 128], F32, tag="pTts")
        nc.vector.memset(pTt_sb[:], 0.5)
        cc_in = dram.tile([16, 128], F32, tag="cci")
        cc_out = dram.tile([128, 128], F32, tag="cco", addr_space="Shared")
        nc.gpsimd.dma_start(cc_in[:], pTt_sb[:])
        nc.gpsimd.collective_compute(
            "AllGather", mybir.AluOpType.bypass, replica_groups=GROUP,
            ins=[cc_in.opt()], outs=[cc_out.opt()],
        )
        if exchange == "agonly":
            g_sb = pool.tile([128, 128], F32, tag="g")
            nc.gpsimd.dma_start(g_sb[:], cc_out[:])
        return

    # ---- load local hidden slice: partition p = rows 4p..4p+3 (16KB contig),
    # split across both HWDGE rings
    # two half-tiles: the first half's chain (fold + pT matmul + exchange
    # kickoff) starts as soon as chunks 0,1 land, even with whole-tile
    # dependency granularity
    h_a = pool.tile([128, 2, D], bdt, tag="ha", bufs=3)
    h_b = pool.tile([128, 2, D], bdt, tag="hb", bufs=3)
    if big_h:
        # probe-only: body it loads a DISTINCT 2MB slice of a 32MB input so
        # no compiler/HW effect can collapse identical bodies
        hap = h_d.ap().rearrange("(i p c) d -> i p c d", i=16, p=128)[it % 16]
    else:
        hap = h_d.ap().rearrange("(p c) d -> p c d", p=128)
    nc.sync.dma_start(h_a[:, 0:1, :], hap[:, 0:1, :])
    nc.scalar.dma_start(h_a[:, 1:2, :], hap[:, 1:2, :])
    nc.sync.dma_start(h_b[:, 0:1, :], hap[:, 2:3, :])
    nc.scalar.dma_start(h_b[:, 1:2, :], hap[:, 3:4, :])

    # ---- fold the 4 rows per partition on DVE (in place: 3 adds)
    # ---- fold rows in TWO halves so the pT matmuls (and so the exchange)
    # can start as soon as the first two DMA chunks land, ~1-2us earlier
    # than waiting for the full slice; the halves sum in PSUM (f32), which
    # also removes one bf16 rounding step vs a 4-way DVE fold
    hsum = pool.tile([128, 2, D], bdt, tag="hsum")
    nc.vector.tensor_tensor(hsum[:, 0, :], h_a[:, 0, :], h_a[:, 1, :],
                            mybir.AluOpType.add)
    nc.vector.tensor_tensor(hsum[:, 1, :], h_b[:, 0, :], h_b[:, 1, :],
                            mybir.AluOpType.add)

    # ---- per-core partial column sums:
    # pT[p, dc*2+b] = (1/S) * sum_{local rows of batch b} h[row, dc*128+p]
    pT_ps = psum1.tile([128, 2 * DC], F32, tag="pT")
    for dc in range(DC):
        for half in range(2):
            # start/stop pairs must be adjacent per PSUM region
            nc.tensor.matmul(
                pT_ps[:, dc * 2:dc * 2 + 2],
                hsum[:, half, dc * 128:(dc + 1) * 128],
                ones2m[:],
                start=(half == 0),
                stop=(half == 1),
            )
    pT_sb = pool.tile([128, 2 * DC], F32, tag="pTs")
    if exchange == "remote" and it >= 3:
        with tc.tile_critical():
            # remote sends of iteration it-2 (remote index it-3) must have
            # drained before reusing pT_sb's buffer (bufs=2)
            nc.vector.wait_ge(nc._ls1, 112 * (it - 2))
            nc.vector.tensor_copy(pT_sb[:], pT_ps[:])
    elif exchange == "remote2" and it >= 3:
        nc.vector.wait_ge(nc._ls1, 112 * (it - 2))
        nc.vector.tensor_copy(pT_sb[:], pT_ps[:])
    else:
        nc.vector.tensor_copy(pT_sb[:], pT_ps[:])

    # ---- cross-core combine -> hbT
    if exchange in ("ag", "ag2", "agalt", "ags", "agb", "agbx"):
        # transpose via identity matmul: pTt[f, q] = pT[q, f]
        xdt = BF16 if exchange == "agbx" else F32
        pTt_ps = psum1.tile([2 * DC, 128], F32, tag="pTt", bufs=1)
        nc.tensor.matmul(pTt_ps[:], pT_sb[:], ident[:], start=True, stop=True)
        pTt_sb = pool.tile([2 * DC, 128], xdt, tag="pTts")
        nc.vector.tensor_copy(pTt_sb[:], pTt_ps[:])
        cc_eng = None
        if exchange == "agalt":
            # alternate the blocking collective between two queues so two
            # AllGathers can be in flight and each queue only stalls every
            # other body
            cc_eng = nc.tensor if (it % 2) else nc.gpsimd
        hbT = _emit_exchange_ag(nc, tc, pool, psum1, dram, pTt_sb, mask16,
                                hwdge=(exchange == "ag2"), cc_eng=cc_eng,
                                xdt=xdt)
    elif exchange in ("none", "noneb"):
        hbT = pT_sb
    elif exchange == "ncfw" or (
            exchange in ("remote", "remote2", "remote3") and it == 0):
        hbT = _emit_exchange_ncfw(nc, tc, pool, dram, pT_sb)
    elif exchange == "remote3":
        hbT = _emit_exchange_remote3(nc, tc, pool, pT_sb, it - 1)
    elif exchange == "remote2":
        hbT = _emit_exchange_remote2(nc, tc, pool, pT_sb, it - 1)
    else:
        hbT = _emit_exchange_remote(nc, tc, pool, pT_sb, it - 1)
    _ITER[0] += 1
    if exchange == "ags":
        return hbT
    _emit_tail(nc, pools, const, out_d, hbT)


def _emit_tail(nc, pools, const, out_d, hbT):
    pool, psum1, psum2, dram = pools
    ones2m, m_sb, mask16, ident = const
    bdt = m_sb.dtype

    # ---- selb[:, dc, :] as lhsT: lhsT[dq, p] = hbT[dq, dc*2 + p//64]
    selb = pool.tile([128, DC, 2, 64], bdt, tag="selb")
    nc.vector.tensor_copy(
        selb[:],
        hbT[:].rearrange("p (dc b) -> p dc b", b=2)
              .unsqueeze(3).broadcast_to([128, DC, 2, 64]),
    )

    # ---- fused r-compute + partition broadcast:
    # bc[p, dout] = sum_d hbar[b(p), d] * M[d, dout] = r[b(p), dout]
    bc_ps = psum2.tile([128, D], F32, tag="bc")
    for half in range(2):
        for dc in range(DC):
            nc.tensor.matmul(
                bc_ps[:, half * 512:(half + 1) * 512],
                selb[:, dc, :, :].rearrange("p b r -> p (b r)"),
                m_sb[:, dc, half * 512:(half + 1) * 512],
                start=(dc == 0),
                stop=(dc == DC - 1),
            )
    bc_sb = pool.tile([128, D], bdt, tag="bcs", bufs=3)
    nc.scalar.copy(bc_sb[:], bc_ps[:])

    # ---- store: out row 4p+c = bc[p, :]  (b(row) = p//64 for all c),
    # split across both HWDGE rings
    oap = out_d.ap().rearrange("(p c) d -> p c d", p=128)
    ds = getattr(nc, "_dsplit", 4)
    if ds == 2:
        nc.sync.dma_start(oap[:, 0:2, :],
                          bc_sb[:].unsqueeze(1).broadcast_to([128, 2, D]))
        nc.scalar.dma_start(oap[:, 2:4, :],
                            bc_sb[:].unsqueeze(1).broadcast_to([128, 2, D]))
    else:
        for c in range(4):
            eng = nc.sync if c % 2 == 0 else nc.scalar
            if ds == 8:
                eng.dma_start(oap[:, c:c + 1, 0:512],
                              bc_sb[:, 0:512].unsqueeze(1)
                              .broadcast_to([128, 1, 512]))
                eng.dma_start(oap[:, c:c + 1, 512:1024],
                              bc_sb[:, 512:1024].unsqueeze(1)
                              .broadcast_to([128, 1, 512]))
            else:
                eng.dma_start(oap[:, c:c + 1, :],
                              bc_sb[:].unsqueeze(1).broadcast_to([128, 1, D]))


def build(loop_k: int = 0, num_devices: int = N_CORES, compile: bool = True,
          exchange: str | None = None, big_h: bool = False,
          nsends: int | None = None, dsplit: int | None = None):
    exchange = EXCHANGE if exchange is None else exchange
    nc = bacc.Bacc("TRN2", target_bir_lowering=False, debug=False,
                   num_devices=num_devices)
    if dsplit is not None:
        nc._dsplit = dsplit
    _ITER[0] = 0
    if exchange in ("remote", "remote2", "remote3"):
        nc._rs1 = nc.alloc_semaphore("rs1")
        nc._ls1 = nc.alloc_semaphore("ls1")
        nc._prep1 = nc.alloc_semaphore("prep1")
        nc._consumed = nc.alloc_semaphore("consumed")
        if exchange == "remote3":
            nc._rs = [nc.alloc_semaphore(f"rs_w{w}") for w in range(4)]
            if nsends is not None:
                nc._nsends = nsends
    nc.has_collectives = True
    h_rows = 16 * ROWS if big_h else ROWS
    bulk_dt = BF16 if exchange in BF_MODES else F32
    h_d = nc.dram_tensor("h", [h_rows, D], bulk_dt, kind="ExternalInput")
    wv_d = nc.dram_tensor("wv", [D, D], F32, kind="ExternalInput")
    wot_d = nc.dram_tensor("wot", [D, D], F32, kind="ExternalInput")
    aux_d = (nc.dram_tensor("aux", [128, 144], F32, kind="ExternalInput")
             if exchange.startswith("ag") else None)
    out_d = nc.dram_tensor("out", [ROWS, D], bulk_dt, kind="ExternalOutput")

    with tile.TileContext(nc) as tc:
        with (
            tc.tile_pool(name="sbuf", bufs=2) as pool,
            tc.tile_pool(name="psum1", bufs=2, space="PSUM") as psum1,
            tc.tile_pool(name="psum2",
                         bufs=1 if exchange.startswith("ag") else 2,
                         space="PSUM") as psum2,
            tc.tile_pool(name="psumm", bufs=1, space="PSUM") as psum_m0,
            tc.tile_pool(name="dram", bufs=4, space="DRAM") as dram,
        ):
            # ag needs PSUM banks for the transpose/reduce tiles: fold the
            # const-time M-fold PSUM into psum2 so the total stays <= 8 banks
            psum_m = psum2 if exchange.startswith("ag") else psum_m0
            const = _emit_const(nc, tc, pool, psum_m, wv_d, wot_d, exchange,
                                aux_d)
            pools = (pool, psum1, psum2, dram)
            if exchange == "agp":
                depth = 2
                pend = []
                for _ in range(max(1, loop_k)):
                    pend.append(_emit_body_a(nc, tc, pools, const, h_d, big_h))
                    if len(pend) > depth:
                        _emit_body_b(nc, tc, pools, const, out_d, pend.pop(0))
                while pend:
                    _emit_body_b(nc, tc, pools, const, out_d, pend.pop(0))
            elif exchange == "ags":
                # stores one body behind: the sync/scalar queues never stall
                # on the current body's AllGather
                pend = []
                for _ in range(max(1, loop_k)):
                    pend.append(_emit_body(nc, tc, pools, const, h_d, out_d,
                                           exchange, big_h))
                    if len(pend) > 1:
                        _emit_tail(nc, pools, const, out_d, pend.pop(0))
                while pend:
                    _emit_tail(nc, pools, const, out_d, pend.pop(0))
            else:
                for _ in range(max(1, loop_k)):
                    _emit_body(nc, tc, pools, const, h_d, out_d, exchange,
                               big_h)
    if compile:
        nc.compile()
    return nc


def _get(loop_k: int = 0):
    key = (loop_k, EXCHANGE)
    if key not in _BUILT:
        _BUILT[key] = build(loop_k)
    return _BUILT[key]


def _aux_const():
    aux = np.zeros((128, 144), np.float32)
    for p in range(128):
        aux[p, p % 16] = 1.0          # mask16
        aux[p, 16 + p] = 1.0          # identity
    return aux


def make_in_maps(hidden_states, Wv, Wo, mode=None):
    mode = EXCHANGE if mode is None else mode
    hidden_states = np.asarray(hidden_states, dtype=np.float32)
    Wv = np.ascontiguousarray(np.asarray(Wv, dtype=np.float32))
    WoT = np.ascontiguousarray(np.asarray(Wo, dtype=np.float32).T)
    aux = _aux_const()
    if mode in BF_MODES:
        import ml_dtypes
        hidden_states = hidden_states.astype(ml_dtypes.bfloat16)
    in_maps = []
    for c in range(N_CORES):
        sl = slice(c * S_LOC, (c + 1) * S_LOC)
        in_maps.append({
            "h": np.ascontiguousarray(hidden_states[:, sl, :]).reshape(ROWS, D),
            "wv": Wv,
            "wot": WoT,
            "aux": aux,
        })
    return in_maps


def assemble(results):
    out = np.empty((B, S, D), np.float32)
    for c in range(N_CORES):
        o = results[c]["out"].reshape(B, S_LOC, D)
        if o.dtype != np.float32:
            o = o.astype(np.float32)
        out[:, c * S_LOC:(c + 1) * S_LOC, :] = o
    return out


def kernel(hidden_states, Wq=None, Wk=None, Wv=None, Wo=None, **_unused):
    nc = _get(0)
    in_maps = make_in_maps(hidden_states, Wv, Wo)
    res = run_bass_kernel_spmd(nc, in_maps, list(range(N_CORES)))
    return assemble(res.results)


if __name__ == "__main__":
    rng = np.random.default_rng(0)
    h = rng.standard_normal((B, S, D), dtype=np.float32)
    wv = rng.standard_normal((D, D), dtype=np.float32) * 0.02
    wo = rng.standard_normal((D, D), dtype=np.float32) * 0.02
    out = kernel(h, None, None, wv, wo)
    ref = (h.mean(axis=1) @ wv.T @ wo.T)[:, None, :] * np.ones((1, S, 1), np.float32)
    err = np.abs(out - ref).max() / np.abs(ref).max()
    print("self-check rel err:", err)


# revision 64
# speedup vs baseline: 1.0163x; 1.0163x over previous
"""Trainium2 Bass kernel for nn_Attention_50989851738305.

The reference module applies jnp.tril(scores, k=-999999) which zeroes the
entire score matrix (S=2048 << 999999), so softmax is uniform 1/S and the
attention output reduces exactly to

    out[b, s, :] = (mean_s' hidden[b, s', :]) @ Wv.T @ Wo.T   (constant in s)

Wq/Wk are mathematically irrelevant. Per core (sequence sharded 8x):

  - Wv/Wo are folded ON DEVICE once per kernel invocation into
    M = Wv.T @ Wo.T (4MB, SBUF-resident), so each iteration needs ONE
    cross-core exchange (8KB of full-sequence partial column sums).
  - The bulk 2MB hidden load and 2MB output store are each split across
    BOTH HWDGE rings (sync + scalar).
  - r[b] = hbar[b] @ M is computed fused with the 128-partition broadcast
    (lhsT column p is hbar[p//64, :], so PSUM row p = r[b(p)]), matching
    the out[(p c)] store pattern where output row 4p+c belongs to batch
    p//64 for every c.

EXCHANGE modes for the 8KB cross-core partial-sum exchange (default "agb",
the shipped configuration; the others are kept for A/B probing):
  "agb":    PRODUCTION: bulk data (hidden load, row sums, M, output store)
            in bf16 -- halves the dominant HBM traffic; partial column
            sums, the exchange, and all matmul accumulation stay f32, so
            measured rel err is ~4.6e-3 against the f32 reference (the
            harness gate is 2e-2). Exchange = firmware AllGather of the
            transposed partials (f-major [16,128] blocks, so the gathered
            [128,128] fetch is contiguous) + one PE mask-matmul that sums
            the 8 rank blocks and lands hbT in the [q, (dc b)] layout the
            tail wants. The 2MB hidden slice loads as two half-tiles
            whose folds feed accumulating PE matmuls (start/stop pairs
            adjacent per PSUM region), so the exchange kickoff only waits
            on half the load. Measured (interleaved paired two-K slope
            diff, distinct-slice bodies): ~7.9us/body in clean windows
            (~13us under heavy shared-machine load) vs ~24.7us for the
            f32 AllReduce baseline and ~8.4us for the f32 exchange-free
            floor; rel err 3.0e-3.
  "ncfw":   f32 + firmware AllReduce (the previous baseline).
  "ag"/"ag2"/"agalt"/"ags"/"agp": f32 AllGather variants (probing).
  "none":   no exchange (numerically wrong, DMA/compute floor probe only).
  "remote"/"remote2"/"remote3": XOR-slot all-gather over
            remote_dma_broadcast (probing; tile_critical overhead makes
            all of them slower than the firmware path here).
"""
import numpy as np

import concourse.bass as bass  # noqa: F401  (bass registers engine types)
import concourse.tile as tile
from concourse import bacc, mybir
from concourse.bass_utils import run_bass_kernel_spmd

B = 2
S = 2048
D = 1024
N_CORES = 8
S_LOC = S // N_CORES          # 256 sequence rows per core (per batch)
ROWS = B * S_LOC              # 512 rows of the local hidden slice
SCALE = 1.0 / S               # uniform attention weight (exact power of two)
F32 = mybir.dt.float32
BF16 = mybir.dt.bfloat16
DC = D // 128                 # 8 chunks of the model dim
GROUP = [list(range(N_CORES))]

EXCHANGE = "agb"              # see docstring
BF_MODES = {"agb", "agbx", "noneb"}   # bulk data in bf16
_BUILT = {}
_ITER = [0]                   # emission counter for cumulative sem targets


def _emit_const(nc, tc, pool, psum_m, wv_d, wot_d, exchange, aux_d):
    """Once per kernel: constants + fold M = Wv.T @ Wo.T (resident, 4MB)."""
    bdt = BF16 if exchange in BF_MODES else F32
    # masked ones: col b has 1/S on partitions where b(p) = p//64 == b
    # (1/2048 is a power of two: exact in bf16)
    ones2m = pool.tile([128, 2], bdt, tag="ones2m", bufs=1)
    nc.vector.memset(ones2m[:], 0.0)
    nc.vector.memset(ones2m[0:64, 0:1], SCALE)
    nc.vector.memset(ones2m[64:128, 1:2], SCALE)

    mask16 = ident = None
    if exchange.startswith("ag"):
        # host-prepared constants (BIR forbids per-partition memset builds):
        # aux[:, 0:16]  = mask16[k*16+f, f'] = 1.0 iff f == f'
        # aux[:, 16:144] = identity for the pT -> pTt PE transpose
        aux = pool.tile([128, 144], F32, tag="aux", bufs=1)
        nc.sync.dma_start(aux[:], aux_d.ap())
        mask16 = aux[:, 0:16]
        ident = aux[:, 16:144]
        if exchange == "agbx":
            # bf16 copy of the reduce mask (PE operands must match dtype)
            mask16b = pool.tile([128, 16], BF16, tag="mask16b", bufs=1)
            nc.vector.tensor_copy(mask16b[:], mask16)
            mask16 = mask16b

    wv_sb = pool.tile([128, DC, D], F32, tag="wv", bufs=1)
    nc.sync.dma_start(wv_sb[:], wv_d.ap().rearrange("(c p) d -> p c d", p=128))
    wot_sb = pool.tile([128, DC, D], F32, tag="wot", bufs=1)
    nc.scalar.dma_start(wot_sb[:], wot_d.ap().rearrange("(c p) d -> p c d", p=128))

    # M[d, dout] = sum_j Wv[j, d] * WoT[j, dout], laid out [p, dc, dout]
    m_sb = pool.tile([128, DC, D], bdt, tag="m", bufs=1)
    for dc in range(DC):
        m_ps = psum_m.tile([128, D], F32, tag="mps", bufs=1)
        for half in range(2):
            for jc in range(DC):
                nc.tensor.matmul(
                    m_ps[:, half * 512:(half + 1) * 512],
                    wv_sb[:, jc, dc * 128:(dc + 1) * 128],
                    wot_sb[:, jc, half * 512:(half + 1) * 512],
                    start=(jc == 0),
                    stop=(jc == DC - 1),
                )
        nc.vector.tensor_copy(m_sb[:, dc, :], m_ps[:])
    return ones2m, m_sb, mask16, ident


def _emit_exchange_ncfw(nc, tc, pool, dram, pT_sb):
    """hbT = AllReduce(pT) over ncfw."""
    cc_in = dram.tile([128, 2 * DC], F32, tag="cci")
    cc_out = dram.tile([128, 2 * DC], F32, tag="cco", addr_space="Shared")
    nc.gpsimd.dma_start(cc_in[:], pT_sb[:])
    nc.gpsimd.collective_compute(
        "AllReduce", mybir.AluOpType.add, replica_groups=GROUP,
        ins=[cc_in.opt()], outs=[cc_out.opt()],
    )
    hbT = pool.tile([128, 2 * DC], F32, tag="hbT")
    nc.gpsimd.dma_start(hbT[:], cc_out[:])
    return hbT


def _emit_exchange_ag(nc, tc, pool, psum_x, dram, pTt_sb, mask16, hwdge,
                      cc_eng=None, xdt=F32):
    """hbT = sum of AllGather'd transposed partials, via one PE matmul.

    pTt_sb: [16, 128] f-major transposed partials (f = dc*2+b, q = d%128).
    AllGather stacks the 8 ranks' 8KB blocks -> cc_out [128, 128] where
    row k*16+f holds rank k's f-row. One matmul with lhsT = gathered tile
    and rhs = mask16 sums over k and transposes back to [q, f].

    hwdge: issue the HBM bounce copies on the sync/scalar queues so the
    gpsimd queue only carries the blocking collective itself.
    """
    cc_in = dram.tile([16, 128], xdt, tag="cci")
    cc_out = dram.tile([128, 128], xdt, tag="cco", addr_space="Shared")
    (nc.sync if hwdge else nc.gpsimd).dma_start(cc_in[:], pTt_sb[:])
    (cc_eng or nc.gpsimd).collective_compute(
        "AllGather", mybir.AluOpType.bypass, replica_groups=GROUP,
        ins=[cc_in.opt()], outs=[cc_out.opt()],
    )
    g_sb = pool.tile([128, 128], xdt, tag="g")
    (nc.scalar if hwdge else nc.gpsimd).dma_start(g_sb[:], cc_out[:])
    hbT_ps = psum_x.tile([128, 2 * DC], F32, tag="hbTps", bufs=1)
    nc.tensor.matmul(hbT_ps[:], g_sb[:], mask16[:], start=True, stop=True)
    hbT = pool.tile([128, 2 * DC], F32, tag="hbT")
    nc.vector.tensor_copy(hbT[:], hbT_ps[:])
    return hbT


def _emit_exchange_remote3(nc, tc, pool, pT_sb, rit):
    """XOR-slot all-gather with ALL exchange ops inside one Pool-only
    tile_critical per body.

    Why this shape:
      - The Tile scheduling sim cannot model remote semaphore increments,
        so manual rs-waits deadlock it OUTSIDE criticals; critical contents
        are exempt.
      - tile_critical drains only the engines used INSIDE it; with
        no_gpsimd_drain=True the Pool drain is a NOP, so DVE/PE/ACT/SP
        never stall and bodies pipeline freely.
      - Criticals chain serially, which is exactly the Pool-serial order
        the exchange needs anyway.

    Slot safety (g bufs=4): sends of round r are ordered after the local
    reduce of r-1 via the consumed gate; a peer's write of round r+4 into
    my slot r%4 requires its reduce r+3, which requires my send r+3, which
    requires my reduce r+2 > r.

    Arrival counting uses one semaphore per slot (rs[w], w = rit%4):
    threshold 14*(rit//4+1). A peer would have to run 4 whole rounds ahead
    to overcount a slot sem, which the consumed gate makes structurally
    impossible -- so the wait implies all 7 peers' round-rit data landed.
    """
    g = pool.tile([128, 8, 2 * DC], F32, tag="g", bufs=4)
    nc.vector.tensor_copy(g[:, 0, :], pT_sb[:])
    w = rit % 4
    nsends = getattr(nc, "_nsends", 7)
    with tc.tile_critical(no_gpsimd_drain=True):
        if rit >= 1:
            nc.gpsimd.wait_ge(nc._consumed, rit)
        for k in range(1, 1 + nsends):
            rdests = [None] * 8
            rdests[k] = (0, k)
            nc.gpsimd.remote_dma_broadcast(
                g[:, k, :], g[:, 0, :],
                remote_sem=nc._rs[w], local_sem=nc._ls1,
                rdests=rdests,
            ).then_inc(nc._prep1, 1)
        nc.gpsimd.wait_ge(nc._prep1, nsends * (rit + 1))
        nc.gpsimd.trigger_dma(nsends)
        nc.gpsimd.wait_ge(nc._rs[w], 2 * nsends * (rit // 4 + 1))
    hbT = pool.tile([128, 2 * DC], F32, tag="hbT")
    nc.vector.tensor_reduce(
        hbT[:], g[:].rearrange("p k f -> p f k"),
        mybir.AxisListType.X, mybir.AluOpType.add,
    )
    # consumed inc rides a nop: the reduce already carries Tile's own sync
    # updates and the TR encoding has no free slot ("Too many sync update
    # commands"); DVE is in-order so nop-completion == reduce done.
    nc.vector.nop(nofuse=True, hint="consumed_inc").then_inc(nc._consumed, 1)
    return hbT


def _emit_exchange_remote2(nc, tc, pool, pT_sb, rit):
    """XOR-slot all-gather without tile_critical (those drain every engine
    at exit -- two per body made the v1 path ~85us/body).

    Slot window: g bufs=4; sends of round r are gated on consumed >= r
    (reduce r-1 done locally). Safety chain: peer's write of round r+4
    into my slot requires peer reduce r+2, which requires my send r+2,
    which requires my reduce r+1 > my reduce r -- so slot r%4 is long
    consumed before it is overwritten. In steady state every wait here is
    pre-satisfied (exchange latency ~3us << round period), so no engine
    stalls."""
    g = pool.tile([128, 8, 2 * DC], F32, tag="g", bufs=4)
    nc.vector.tensor_copy(g[:, 0, :], pT_sb[:])
    if rit >= 1:
        nc.gpsimd.wait_ge(nc._consumed, rit)
    for k in range(1, 8):
        rdests = [None] * 8
        rdests[k] = (0, k)
        nc.gpsimd.remote_dma_broadcast(
            g[:, k, :], pT_sb[:],
            remote_sem=nc._rs1, local_sem=nc._ls1,
            rdests=rdests,
        ).then_inc(nc._prep1, 1)
    nc.gpsimd.wait_ge(nc._prep1, 7 * (rit + 1))
    nc.gpsimd.trigger_dma(7)
    hbT = pool.tile([128, 2 * DC], F32, tag="hbT")
    nc.vector.wait_ge(nc._rs1, 14 * (rit + 1))
    nc.vector.tensor_reduce(
        hbT[:], g[:].rearrange("p k f -> p f k"),
        mybir.AxisListType.X, mybir.AluOpType.add,
    ).then_inc(nc._consumed, 1)
    return hbT


def _emit_exchange_remote(nc, tc, pool, pT_sb, rit):
    """hbT = sum over the 8 cores' pT via XOR-slot all-gather + DVE reduce.

    rit is the 0-based REMOTE iteration index (iteration 0 of the program
    always goes through ncfw, whose firmware rendezvous guarantees every
    core is executing -- with per-execution-zeroed semaphores -- before the
    first remote send fires)."""
    g = pool.tile([128, 8, 2 * DC], F32, tag="g", bufs=4)
    nc.vector.tensor_copy(g[:, 0, :], pT_sb[:])
    with tc.tile_critical():
        if rit >= 1:
            # send(rit) >= consume(rit-1) bounds inter-core skew
            nc.gpsimd.wait_ge(nc._consumed, rit)
        for k in range(1, 8):
            rdests = [None] * 8
            rdests[k] = (0, k)
            nc.gpsimd.remote_dma_broadcast(
                g[:, k, :], pT_sb[:],
                remote_sem=nc._rs1, local_sem=nc._ls1,
                rdests=rdests,
            ).then_inc(nc._prep1, 1)
        nc.gpsimd.wait_ge(nc._prep1, 7 * (rit + 1))
        nc.gpsimd.trigger_dma(7)
    hbT = pool.tile([128, 2 * DC], F32, tag="hbT")
    with tc.tile_critical():
        nc.vector.wait_ge(nc._rs1, 14 * (rit + 1))
        nc.vector.tensor_reduce(
            hbT[:], g[:].rearrange("p k f -> p f k"),
            mybir.AxisListType.X, mybir.AluOpType.add,
        ).then_inc(nc._consumed, 1)
    return hbT


def _emit_body_a(nc, tc, pools, const, h_d, big_h):
    """Pipelined-AG stage A: load + partial sums + AllGather doorbell.

    Returns the ctx stage B needs. The gpsimd queue only carries the 8KB
    cc_in store and the PTC doorbell here -- the PTC releases the sequencer
    at doorbell time (collective runs on the TOPSP cores), so consecutive
    bodies' AllGathers overlap once the fetch is deferred to stage B.
    """
    pool, psum1, psum2, dram = pools
    ones2m, m_sb, mask16, ident = const
    it = _ITER[0]
    _ITER[0] += 1

    h_sb = pool.tile([128, 4, D], F32, tag="h", bufs=4)
    if big_h:
        hap = h_d.ap().rearrange("(i p c) d -> i p c d", i=16, p=128)[it % 16]
    else:
        hap = h_d.ap().rearrange("(p c) d -> p c d", p=128)
    nc.sync.dma_start(h_sb[:, 0:1, :], hap[:, 0:1, :])
    nc.scalar.dma_start(h_sb[:, 1:2, :], hap[:, 1:2, :])
    nc.sync.dma_start(h_sb[:, 2:3, :], hap[:, 2:3, :])
    nc.scalar.dma_start(h_sb[:, 3:4, :], hap[:, 3:4, :])

    hsum = pool.tile([128, D], F32, tag="hsum")
    nc.vector.tensor_tensor(hsum[:], h_sb[:, 0, :], h_sb[:, 1, :],
                            mybir.AluOpType.add)
    nc.vector.tensor_tensor(hsum[:], hsum[:], h_sb[:, 2, :],
                            mybir.AluOpType.add)
    nc.vector.tensor_tensor(hsum[:], hsum[:], h_sb[:, 3, :],
                            mybir.AluOpType.add)

    pT_ps = psum1.tile([128, 2 * DC], F32, tag="pT")
    for dc in range(DC):
        nc.tensor.matmul(
            pT_ps[:, dc * 2:dc * 2 + 2],
            hsum[:, dc * 128:(dc + 1) * 128],
            ones2m[:],
            start=True,
            stop=True,
        )
    pT_sb = pool.tile([128, 2 * DC], F32, tag="pTs")
    nc.vector.tensor_copy(pT_sb[:], pT_ps[:])

    pTt_ps = psum1.tile([2 * DC, 128], F32, tag="pTt", bufs=1)
    nc.tensor.matmul(pTt_ps[:], pT_sb[:], ident[:], start=True, stop=True)
    pTt_sb = pool.tile([2 * DC, 128], F32, tag="pTts")
    nc.vector.tensor_copy(pTt_sb[:], pTt_ps[:])

    cc_in = dram.tile([16, 128], F32, tag="cci")
    cc_out = dram.tile([128, 128], F32, tag="cco", addr_space="Shared")
    nc.gpsimd.dma_start(cc_in[:], pTt_sb[:])
    nc.gpsimd.collective_compute(
        "AllGather", mybir.AluOpType.bypass, replica_groups=GROUP,
        ins=[cc_in.opt()], outs=[cc_out.opt()],
    )
    return cc_out


def _emit_body_b(nc, tc, pools, const, out_d, cc_out):
    """Pipelined-AG stage B: fetch gathered partials + compute + store."""
    pool, psum1, psum2, dram = pools
    ones2m, m_sb, mask16, ident = const

    g_sb = pool.tile([128, 128], F32, tag="g")
    nc.gpsimd.dma_start(g_sb[:], cc_out[:])
    hbT_ps = psum1.tile([128, 2 * DC], F32, tag="hbTps", bufs=1)
    nc.tensor.matmul(hbT_ps[:], g_sb[:], mask16[:], start=True, stop=True)
    hbT = pool.tile([128, 2 * DC], F32, tag="hbT")
    nc.vector.tensor_copy(hbT[:], hbT_ps[:])

    selb = pool.tile([128, DC, 2, 64], F32, tag="selb")
    nc.vector.tensor_copy(
        selb[:],
        hbT[:].rearrange("p (dc b) -> p dc b", b=2)
              .unsqueeze(3).broadcast_to([128, DC, 2, 64]),
    )

    bc_ps = psum2.tile([128, D], F32, tag="bc")
    for half in range(2):
        for dc in range(DC):
            nc.tensor.matmul(
                bc_ps[:, half * 512:(half + 1) * 512],
                selb[:, dc, :, :].rearrange("p b r -> p (b r)"),
                m_sb[:, dc, half * 512:(half + 1) * 512],
                start=(dc == 0),
                stop=(dc == DC - 1),
            )
    bc_sb = pool.tile([128, D], bdt, tag="bcs", bufs=3)
    nc.scalar.copy(bc_sb[:], bc_ps[:])

    oap = out_d.ap().rearrange("(p c) d -> p c d", p=128)
    for c in range(4):
        eng = nc.sync if c % 2 == 0 else nc.scalar
        eng.dma_start(oap[:, c:c + 1, :],
                      bc_sb[:].unsqueeze(1).broadcast_to([128, 1, D]))


def _emit_body(nc, tc, pools, const, h_d, out_d, exchange, big_h):
    pool, psum1, psum2, dram = pools
    ones2m, m_sb, mask16, ident = const
    it = _ITER[0]
    bdt = BF16 if exchange in BF_MODES else F32

    if exchange in ("agonly", "agburst"):
        # exchange-chain-only probes: "agonly" = serial store+AG+fetch per
        # body (latency), "agburst" = store+doorbell only, 4-deep window
        # (ncfw AllGather throughput)
        _ITER[0] += 1
        pTt_sb = pool.tile([16, 128], F32, tag="pTts")
        nc.vector.memset(pTt_sb[:], 0.5)
        cc_in = dram.tile([16, 128], F32, tag="cci")
        cc_out = dram.tile([128, 128], F32, tag="cco", addr_space="Shared")
        nc.gpsimd.dma_start(cc_in[:], pTt_sb[:])
        nc.gpsimd.collective_compute(
            "AllGather", mybir.AluOpType.bypass, replica_groups=GROUP,
            ins=[cc_in.opt()], outs=[cc_out.opt()],
        )
        if exchange == "agonly":
            g_sb = pool.tile([128, 128], F32, tag="g")
            nc.gpsimd.dma_start(g_sb[:], cc_out[:])
        return

    # ---- load local hidden slice: partition p = rows 4p..4p+3 (16KB contig),
    # split across both HWDGE rings
    # two half-tiles: the first half's chain (fold + pT matmul + exchange
    # kickoff) starts as soon as chunks 0,1 land, even with whole-tile
    # dependency granularity
    h_a = pool.tile([128, 2, D], bdt, tag="ha", bufs=3)
    h_b = pool.tile([128, 2, D], bdt, tag="hb", bufs=3)
    if big_h:
        # probe-only: body it loads a DISTINCT 2MB slice of a 32MB input so
        # no compiler/HW effect can collapse identical bodies
        hap = h_d.ap().rearrange("(i p c) d -> i p c d", i=16, p=128)[it % 16]
    else:
        hap = h_d.ap().rearrange("(p c) d -> p c d", p=128)
    ds = getattr(nc, "_dsplit", 5)
    if ds == 5:
        # direction-split queues: ALL loads on sync, ALL stores on scalar.
        # Loads have no AllGather dependency, so the load queue prefetches
        # straight through the per-body exchange stall that otherwise
        # blocks loads queued behind AG-dependent stores.
        nc.sync.dma_start(h_a[:], hap[:, 0:2, :])
        nc.sync.dma_start(h_b[:], hap[:, 2:4, :])
    else:
        nc.sync.dma_start(h_a[:, 0:1, :], hap[:, 0:1, :])
        nc.scalar.dma_start(h_a[:, 1:2, :], hap[:, 1:2, :])
        nc.sync.dma_start(h_b[:, 0:1, :], hap[:, 2:3, :])
        nc.scalar.dma_start(h_b[:, 1:2, :], hap[:, 3:4, :])

    # ---- fold the 4 rows per partition on DVE (in place: 3 adds)
    # ---- fold rows in TWO halves so the pT matmuls (and so the exchange)
    # can start as soon as the first two DMA chunks land, ~1-2us earlier
    # than waiting for the full slice; the halves sum in PSUM (f32), which
    # also removes one bf16 rounding step vs a 4-way DVE fold
    hsum = pool.tile([128, 2, D], bdt, tag="hsum")
    nc.vector.tensor_tensor(hsum[:, 0, :], h_a[:, 0, :], h_a[:, 1, :],
                            mybir.AluOpType.add)
    nc.vector.tensor_tensor(hsum[:, 1, :], h_b[:, 0, :], h_b[:, 1, :],
                            mybir.AluOpType.add)

    # ---- per-core partial column sums:
    # pT[p, dc*2+b] = (1/S) * sum_{local rows of batch b} h[row, dc*128+p]
    pT_ps = psum1.tile([128, 2 * DC], F32, tag="pT")
    for dc in range(DC):
        for half in range(2):
            # start/stop pairs must be adjacent per PSUM region
            nc.tensor.matmul(
                pT_ps[:, dc * 2:dc * 2 + 2],
                hsum[:, half, dc * 128:(dc + 1) * 128],
                ones2m[:],
                start=(half == 0),
                stop=(half == 1),
            )
    pT_sb = pool.tile([128, 2 * DC], F32, tag="pTs")
    if exchange == "remote" and it >= 3:
        with tc.tile_critical():
            # remote sends of iteration it-2 (remote index it-3) must have
            # drained before reusing pT_sb's buffer (bufs=2)
            nc.vector.wait_ge(nc._ls1, 112 * (it - 2))
            nc.vector.tensor_copy(pT_sb[:], pT_ps[:])
    elif exchange == "remote2" and it >= 3:
        nc.vector.wait_ge(nc._ls1, 112 * (it - 2))
        nc.vector.tensor_copy(pT_sb[:], pT_ps[:])
    else:
        nc.vector.tensor_copy(pT_sb[:], pT_ps[:])

    # ---- cross-core combine -> hbT
    if exchange in ("ag", "ag2", "agalt", "ags", "agb", "agbx"):
        # transpose via identity matmul: pTt[f, q] = pT[q, f]
        xdt = BF16 if exchange == "agbx" else F32
        pTt_ps = psum1.tile([2 * DC, 128], F32, tag="pTt", bufs=1)
        nc.tensor.matmul(pTt_ps[:], pT_sb[:], ident[:], start=True, stop=True)
        pTt_sb = pool.tile([2 * DC, 128], xdt, tag="pTts")
        nc.vector.tensor_copy(pTt_sb[:], pTt_ps[:])
        cc_eng = None
        if exchange == "agalt":
            # alternate the blocking collective between two queues so two
            # AllGathers can be in flight and each queue only stalls every
            # other body
            cc_eng = nc.tensor if (it % 2) else nc.gpsimd
        hbT = _emit_exchange_ag(nc, tc, pool, psum1, dram, pTt_sb, mask16,
                                hwdge=(exchange == "ag2"), cc_eng=cc_eng,
                                xdt=xdt)
    elif exchange in ("none", "noneb"):
        hbT = pT_sb
    elif exchange == "ncfw" or (
            exchange in ("remote", "remote2", "remote3") and it == 0):
        hbT = _emit_exchange_ncfw(nc, tc, pool, dram, pT_sb)
    elif exchange == "remote3":
        hbT = _emit_exchange_remote3(nc, tc, pool, pT_sb, it - 1)
    elif exchange == "remote2":
        hbT = _emit_exchange_remote2(nc, tc, pool, pT_sb, it - 1)
    else:
        hbT = _emit_exchange_remote(nc, tc, pool, pT_sb, it - 1)
    _ITER[0] += 1
    if exchange == "ags":
        return hbT
    _emit_tail(nc, pools, const, out_d, hbT)


def _emit_tail(nc, pools, const, out_d, hbT):
    pool, psum1, psum2, dram = pools
    ones2m, m_sb, mask16, ident = const
    bdt = m_sb.dtype

    # ---- selb[:, dc, :] as lhsT: lhsT[dq, p] = hbT[dq, dc*2 + p//64]
    selb = pool.tile([128, DC, 2, 64], bdt, tag="selb")
    nc.vector.tensor_copy(
        selb[:],
        hbT[:].rearrange("p (dc b) -> p dc b", b=2)
              .unsqueeze(3).broadcast_to([128, DC, 2, 64]),
    )

    # ---- fused r-compute + partition broadcast:
    # bc[p, dout] = sum_d hbar[b(p), d] * M[d, dout] = r[b(p), dout]
    bc_ps = psum2.tile([128, D], F32, tag="bc")
    for half in range(2):
        for dc in range(DC):
            nc.tensor.matmul(
                bc_ps[:, half * 512:(half + 1) * 512],
                selb[:, dc, :, :].rearrange("p b r -> p (b r)"),
                m_sb[:, dc, half * 512:(half + 1) * 512],
                start=(dc == 0),
                stop=(dc == DC - 1),
            )
    bc_sb = pool.tile([128, D], bdt, tag="bcs", bufs=3)
    nc.scalar.copy(bc_sb[:], bc_ps[:])

    # ---- store: out row 4p+c = bc[p, :]  (b(row) = p//64 for all c),
    # split across both HWDGE rings
    oap = out_d.ap().rearrange("(p c) d -> p c d", p=128)
    ds = getattr(nc, "_dsplit", 5)
    if ds == 5:
        nc.scalar.dma_start(oap[:, 0:2, :],
                            bc_sb[:].unsqueeze(1).broadcast_to([128, 2, D]))
        nc.scalar.dma_start(oap[:, 2:4, :],
                            bc_sb[:].unsqueeze(1).broadcast_to([128, 2, D]))
    elif ds == 2:
        nc.sync.dma_start(oap[:, 0:2, :],
                          bc_sb[:].unsqueeze(1).broadcast_to([128, 2, D]))
        nc.scalar.dma_start(oap[:, 2:4, :],
                            bc_sb[:].unsqueeze(1).broadcast_to([128, 2, D]))
    else:
        for c in range(4):
            eng = nc.sync if c % 2 == 0 else nc.scalar
            if ds == 8:
                eng.dma_start(oap[:, c:c + 1, 0:512],
                              bc_sb[:, 0:512].unsqueeze(1)
                              .broadcast_to([128, 1, 512]))
                eng.dma_start(oap[:, c:c + 1, 512:1024],
                              bc_sb[:, 512:1024].unsqueeze(1)
                              .broadcast_to([128, 1, 512]))
            else:
                eng.dma_start(oap[:, c:c + 1, :],
                              bc_sb[:].unsqueeze(1).broadcast_to([128, 1, D]))


def build(loop_k: int = 0, num_devices: int = N_CORES, compile: bool = True,
          exchange: str | None = None, big_h: bool = False,
          nsends: int | None = None, dsplit: int | None = None):
    exchange = EXCHANGE if exchange is None else exchange
    nc = bacc.Bacc("TRN2", target_bir_lowering=False, debug=False,
                   num_devices=num_devices)
    if dsplit is not None:
        nc._dsplit = dsplit
    _ITER[0] = 0
    if exchange in ("remote", "remote2", "remote3"):
        nc._rs1 = nc.alloc_semaphore("rs1")
        nc._ls1 = nc.alloc_semaphore("ls1")
        nc._prep1 = nc.alloc_semaphore("prep1")
        nc._consumed = nc.alloc_semaphore("consumed")
        if exchange == "remote3":
            nc._rs = [nc.alloc_semaphore(f"rs_w{w}") for w in range(4)]
            if nsends is not None:
                nc._nsends = nsends
    nc.has_collectives = True
    h_rows = 16 * ROWS if big_h else ROWS
    bulk_dt = BF16 if exchange in BF_MODES else F32
    h_d = nc.dram_tensor("h", [h_rows, D], bulk_dt, kind="ExternalInput")
    wv_d = nc.dram_tensor("wv", [D, D], F32, kind="ExternalInput")
    wot_d = nc.dram_tensor("wot", [D, D], F32, kind="ExternalInput")
    aux_d = (nc.dram_tensor("aux", [128, 144], F32, kind="ExternalInput")
             if exchange.startswith("ag") else None)
    out_d = nc.dram_tensor("out", [ROWS, D], bulk_dt, kind="ExternalOutput")

    with tile.TileContext(nc) as tc:
        with (
            tc.tile_pool(name="sbuf", bufs=2) as pool,
            tc.tile_pool(name="psum1", bufs=2, space="PSUM") as psum1,
            tc.tile_pool(name="psum2",
                         bufs=1 if exchange.startswith("ag") else 2,
                         space="PSUM") as psum2,
            tc.tile_pool(name="psumm", bufs=1, space="PSUM") as psum_m0,
            tc.tile_pool(name="dram", bufs=4, space="DRAM") as dram,
        ):
            # ag needs PSUM banks for the transpose/reduce tiles: fold the
            # const-time M-fold PSUM into psum2 so the total stays <= 8 banks
            psum_m = psum2 if exchange.startswith("ag") else psum_m0
            const = _emit_const(nc, tc, pool, psum_m, wv_d, wot_d, exchange,
                                aux_d)
            pools = (pool, psum1, psum2, dram)
            if exchange == "agp":
                depth = 2
                pend = []
                for _ in range(max(1, loop_k)):
                    pend.append(_emit_body_a(nc, tc, pools, const, h_d, big_h))
                    if len(pend) > depth:
                        _emit_body_b(nc, tc, pools, const, out_d, pend.pop(0))
                while pend:
                    _emit_body_b(nc, tc, pools, const, out_d, pend.pop(0))
            elif exchange == "ags":
                # stores one body behind: the sync/scalar queues never stall
                # on the current body's AllGather
                pend = []
                for _ in range(max(1, loop_k)):
                    pend.append(_emit_body(nc, tc, pools, const, h_d, out_d,
                                           exchange, big_h))
                    if len(pend) > 1:
                        _emit_tail(nc, pools, const, out_d, pend.pop(0))
                while pend:
                    _emit_tail(nc, pools, const, out_d, pend.pop(0))
            else:
                for _ in range(max(1, loop_k)):
                    _emit_body(nc, tc, pools, const, h_d, out_d, exchange,
                               big_h)
    if compile:
        nc.compile()
    return nc


def _get(loop_k: int = 0):
    key = (loop_k, EXCHANGE)
    if key not in _BUILT:
        _BUILT[key] = build(loop_k)
    return _BUILT[key]


def _aux_const():
    aux = np.zeros((128, 144), np.float32)
    for p in range(128):
        aux[p, p % 16] = 1.0          # mask16
        aux[p, 16 + p] = 1.0          # identity
    return aux


def make_in_maps(hidden_states, Wv, Wo, mode=None):
    mode = EXCHANGE if mode is None else mode
    hidden_states = np.asarray(hidden_states, dtype=np.float32)
    Wv = np.ascontiguousarray(np.asarray(Wv, dtype=np.float32))
    WoT = np.ascontiguousarray(np.asarray(Wo, dtype=np.float32).T)
    aux = _aux_const()
    if mode in BF_MODES:
        import ml_dtypes
        hidden_states = hidden_states.astype(ml_dtypes.bfloat16)
    in_maps = []
    for c in range(N_CORES):
        sl = slice(c * S_LOC, (c + 1) * S_LOC)
        in_maps.append({
            "h": np.ascontiguousarray(hidden_states[:, sl, :]).reshape(ROWS, D),
            "wv": Wv,
            "wot": WoT,
            "aux": aux,
        })
    return in_maps


def assemble(results):
    out = np.empty((B, S, D), np.float32)
    for c in range(N_CORES):
        o = results[c]["out"].reshape(B, S_LOC, D)
        if o.dtype != np.float32:
            o = o.astype(np.float32)
        out[:, c * S_LOC:(c + 1) * S_LOC, :] = o
    return out


def kernel(hidden_states, Wq=None, Wk=None, Wv=None, Wo=None, **_unused):
    nc = _get(0)
    in_maps = make_in_maps(hidden_states, Wv, Wo)
    res = run_bass_kernel_spmd(nc, in_maps, list(range(N_CORES)))
    return assemble(res.results)


if __name__ == "__main__":
    rng = np.random.default_rng(0)
    h = rng.standard_normal((B, S, D), dtype=np.float32)
    wv = rng.standard_normal((D, D), dtype=np.float32) * 0.02
    wo = rng.standard_normal((D, D), dtype=np.float32) * 0.02
    out = kernel(h, None, None, wv, wo)
    ref = (h.mean(axis=1) @ wv.T @ wo.T)[:, None, :] * np.ones((1, S, 1), np.float32)
    err = np.abs(out - ref).max() / np.abs(ref).max()
    print("self-check rel err:", err)


# revision 67
# speedup vs baseline: 1.8121x; 1.7830x over previous
"""Trainium2 Bass kernel for nn_Attention_50989851738305.

The reference module applies jnp.tril(scores, k=-999999) which zeroes the
entire score matrix (S=2048 << 999999), so softmax is uniform 1/S and the
attention output reduces exactly to

    out[b, s, :] = (mean_s' hidden[b, s', :]) @ Wv.T @ Wo.T   (constant in s)

Wq/Wk are mathematically irrelevant. Per core (sequence sharded 8x):

  - Wv/Wo are folded ON DEVICE once per kernel invocation into
    M = Wv.T @ Wo.T (4MB, SBUF-resident), so each iteration needs ONE
    cross-core exchange (8KB of full-sequence partial column sums).
  - Bulk DMA queues are DIRECTION-SPLIT: all hidden loads on the sync
    HWDGE ring, all output stores on scalar. Stores depend on the per-body
    AllGather; homogeneous queues let the load stream prefetch straight
    through the exchange stall instead of queuing behind stalled stores
    (won 6/6 interleaved paired reps vs per-direction interleaving).
  - r[b] = hbar[b] @ M is computed fused with the 128-partition broadcast
    (lhsT column p is hbar[p//64, :], so PSUM row p = r[b(p)]), matching
    the out[(p c)] store pattern where output row 4p+c belongs to batch
    p//64 for every c.

EXCHANGE modes for the 8KB cross-core partial-sum exchange (default "agb",
the shipped configuration; the others are kept for A/B probing):
  "agb":    PRODUCTION: bulk data (hidden load, row sums, M, output store)
            in bf16 -- halves the dominant HBM traffic; partial column
            sums, the exchange, and all matmul accumulation stay f32, so
            measured rel err is ~4.6e-3 against the f32 reference (the
            harness gate is 2e-2). Exchange = firmware AllGather of the
            transposed partials (f-major [16,128] blocks, so the gathered
            [128,128] fetch is contiguous) + one PE mask-matmul that sums
            the 8 rank blocks and lands hbT in the [q, (dc b)] layout the
            tail wants. The 2MB hidden slice loads as two half-tiles
            whose folds feed accumulating PE matmuls (start/stop pairs
            adjacent per PSUM region), so the exchange kickoff only waits
            on half the load. Measured (interleaved paired two-K slope
            diff, distinct-slice bodies): ~8us/body in clean windows
            (~11.6-13us under heavy shared-machine load) vs ~24.7us for
            the f32 AllReduce baseline and ~8.4us for the f32
            exchange-free floor; rel err 3.0e-3. Exchange bounce DMAs
            stay on gpsimd: routing them to the HWDGE queues (xq knob)
            lost 5/6 paired reps even with direction-split bulk.
  "ncfw":   f32 + firmware AllReduce (the previous baseline).
  "ag"/"ag2"/"agalt"/"ags"/"agp": f32 AllGather variants (probing).
  "none":   no exchange (numerically wrong, DMA/compute floor probe only).
  "remote"/"remote2"/"remote3": XOR-slot all-gather over
            remote_dma_broadcast (probing; tile_critical overhead makes
            all of them slower than the firmware path here).
"""
import numpy as np

import concourse.bass as bass  # noqa: F401  (bass registers engine types)
import concourse.tile as tile
from concourse import bacc, mybir
from concourse.bass_utils import run_bass_kernel_spmd

B = 2
S = 2048
D = 1024
N_CORES = 8
S_LOC = S // N_CORES          # 256 sequence rows per core (per batch)
ROWS = B * S_LOC              # 512 rows of the local hidden slice
SCALE = 1.0 / S               # uniform attention weight (exact power of two)
F32 = mybir.dt.float32
BF16 = mybir.dt.bfloat16
DC = D // 128                 # 8 chunks of the model dim
GROUP = [list(range(N_CORES))]

EXCHANGE = "agb"              # see docstring
BF_MODES = {"agb", "agbx", "noneb"}   # bulk data in bf16
_BUILT = {}
_ITER = [0]                   # emission counter for cumulative sem targets


def _emit_const(nc, tc, pool, psum_m, wv_d, wot_d, exchange, aux_d):
    """Once per kernel: constants + fold M = Wv.T @ Wo.T (resident, 4MB)."""
    bdt = BF16 if exchange in BF_MODES else F32
    # masked ones: col b has 1/S on partitions where b(p) = p//64 == b
    # (1/2048 is a power of two: exact in bf16)
    ones2m = pool.tile([128, 2], bdt, tag="ones2m", bufs=1)
    nc.vector.memset(ones2m[:], 0.0)
    nc.vector.memset(ones2m[0:64, 0:1], SCALE)
    nc.vector.memset(ones2m[64:128, 1:2], SCALE)

    mask16 = ident = None
    if exchange.startswith("ag"):
        # host-prepared constants (BIR forbids per-partition memset builds):
        # aux[:, 0:16]  = mask16[k*16+f, f'] = 1.0 iff f == f'
        # aux[:, 16:144] = identity for the pT -> pTt PE transpose
        aux = pool.tile([128, 144], F32, tag="aux", bufs=1)
        nc.sync.dma_start(aux[:], aux_d.ap())
        mask16 = aux[:, 0:16]
        ident = aux[:, 16:144]
        if exchange == "agbx":
            # bf16 copy of the reduce mask (PE operands must match dtype)
            mask16b = pool.tile([128, 16], BF16, tag="mask16b", bufs=1)
            nc.vector.tensor_copy(mask16b[:], mask16)
            mask16 = mask16b

    wv_sb = pool.tile([128, DC, D], F32, tag="wv", bufs=1)
    nc.sync.dma_start(wv_sb[:], wv_d.ap().rearrange("(c p) d -> p c d", p=128))
    wot_sb = pool.tile([128, DC, D], F32, tag="wot", bufs=1)
    nc.scalar.dma_start(wot_sb[:], wot_d.ap().rearrange("(c p) d -> p c d", p=128))

    # M[d, dout] = sum_j Wv[j, d] * WoT[j, dout], laid out [p, dc, dout]
    m_sb = pool.tile([128, DC, D], bdt, tag="m", bufs=1)
    for dc in range(DC):
        m_ps = psum_m.tile([128, D], F32, tag="mps", bufs=1)
        for half in range(2):
            for jc in range(DC):
                nc.tensor.matmul(
                    m_ps[:, half * 512:(half + 1) * 512],
                    wv_sb[:, jc, dc * 128:(dc + 1) * 128],
                    wot_sb[:, jc, half * 512:(half + 1) * 512],
                    start=(jc == 0),
                    stop=(jc == DC - 1),
                )
        nc.vector.tensor_copy(m_sb[:, dc, :], m_ps[:])
    return ones2m, m_sb, mask16, ident


def _emit_exchange_ncfw(nc, tc, pool, dram, pT_sb):
    """hbT = AllReduce(pT) over ncfw."""
    cc_in = dram.tile([128, 2 * DC], F32, tag="cci")
    cc_out = dram.tile([128, 2 * DC], F32, tag="cco", addr_space="Shared")
    nc.gpsimd.dma_start(cc_in[:], pT_sb[:])
    nc.gpsimd.collective_compute(
        "AllReduce", mybir.AluOpType.add, replica_groups=GROUP,
        ins=[cc_in.opt()], outs=[cc_out.opt()],
    )
    hbT = pool.tile([128, 2 * DC], F32, tag="hbT")
    nc.gpsimd.dma_start(hbT[:], cc_out[:])
    return hbT


def _emit_exchange_ag(nc, tc, pool, psum_x, dram, pTt_sb, mask16, hwdge,
                      cc_eng=None, xdt=F32):
    """hbT = sum of AllGather'd transposed partials, via one PE matmul.

    pTt_sb: [16, 128] f-major transposed partials (f = dc*2+b, q = d%128).
    AllGather stacks the 8 ranks' 8KB blocks -> cc_out [128, 128] where
    row k*16+f holds rank k's f-row. One matmul with lhsT = gathered tile
    and rhs = mask16 sums over k and transposes back to [q, f].

    hwdge: issue the HBM bounce copies on the sync/scalar queues so the
    gpsimd queue only carries the blocking collective itself.
    """
    # xq routing (only sensible with direction-split bulk queues, where
    # scalar carries AG-dependent stores anyway): 1 = fetch on scalar,
    # 2 = also bounce store on sync -- gpsimd then only doorbells, so
    # consecutive AllGathers queue at TOPSP instead of serializing on the
    # fetch's completion wait
    xq = getattr(nc, "_xq", 0)
    cc_in = dram.tile([16, 128], xdt, tag="cci")
    cc_out = dram.tile([128, 128], xdt, tag="cco", addr_space="Shared")
    store_eng = nc.sync if (hwdge or xq >= 2) else nc.gpsimd
    fetch_eng = nc.scalar if (hwdge or xq >= 1) else nc.gpsimd
    store_eng.dma_start(cc_in[:], pTt_sb[:])
    (cc_eng or nc.gpsimd).collective_compute(
        "AllGather", mybir.AluOpType.bypass, replica_groups=GROUP,
        ins=[cc_in.opt()], outs=[cc_out.opt()],
    )
    g_sb = pool.tile([128, 128], xdt, tag="g")
    fetch_eng.dma_start(g_sb[:], cc_out[:])
    hbT_ps = psum_x.tile([128, 2 * DC], F32, tag="hbTps", bufs=1)
    nc.tensor.matmul(hbT_ps[:], g_sb[:], mask16[:], start=True, stop=True)
    hbT = pool.tile([128, 2 * DC], F32, tag="hbT")
    nc.vector.tensor_copy(hbT[:], hbT_ps[:])
    return hbT


def _emit_exchange_remote3(nc, tc, pool, pT_sb, rit):
    """XOR-slot all-gather with ALL exchange ops inside one Pool-only
    tile_critical per body.

    Why this shape:
      - The Tile scheduling sim cannot model remote semaphore increments,
        so manual rs-waits deadlock it OUTSIDE criticals; critical contents
        are exempt.
      - tile_critical drains only the engines used INSIDE it; with
        no_gpsimd_drain=True the Pool drain is a NOP, so DVE/PE/ACT/SP
        never stall and bodies pipeline freely.
      - Criticals chain serially, which is exactly the Pool-serial order
        the exchange needs anyway.

    Slot safety (g bufs=4): sends of round r are ordered after the local
    reduce of r-1 via the consumed gate; a peer's write of round r+4 into
    my slot r%4 requires its reduce r+3, which requires my send r+3, which
    requires my reduce r+2 > r.

    Arrival counting uses one semaphore per slot (rs[w], w = rit%4):
    threshold 14*(rit//4+1). A peer would have to run 4 whole rounds ahead
    to overcount a slot sem, which the consumed gate makes structurally
    impossible -- so the wait implies all 7 peers' round-rit data landed.
    """
    g = pool.tile([128, 8, 2 * DC], F32, tag="g", bufs=4)
    nc.vector.tensor_copy(g[:, 0, :], pT_sb[:])
    w = rit % 4
    nsends = getattr(nc, "_nsends", 7)
    with tc.tile_critical(no_gpsimd_drain=True):
        if rit >= 1:
            nc.gpsimd.wait_ge(nc._consumed, rit)
        for k in range(1, 1 + nsends):
            rdests = [None] * 8
            rdests[k] = (0, k)
            nc.gpsimd.remote_dma_broadcast(
                g[:, k, :], g[:, 0, :],
                remote_sem=nc._rs[w], local_sem=nc._ls1,
                rdests=rdests,
            ).then_inc(nc._prep1, 1)
        nc.gpsimd.wait_ge(nc._prep1, nsends * (rit + 1))
        nc.gpsimd.trigger_dma(nsends)
        nc.gpsimd.wait_ge(nc._rs[w], 2 * nsends * (rit // 4 + 1))
    hbT = pool.tile([128, 2 * DC], F32, tag="hbT")
    nc.vector.tensor_reduce(
        hbT[:], g[:].rearrange("p k f -> p f k"),
        mybir.AxisListType.X, mybir.AluOpType.add,
    )
    # consumed inc rides a nop: the reduce already carries Tile's own sync
    # updates and the TR encoding has no free slot ("Too many sync update
    # commands"); DVE is in-order so nop-completion == reduce done.
    nc.vector.nop(nofuse=True, hint="consumed_inc").then_inc(nc._consumed, 1)
    return hbT


def _emit_exchange_remote2(nc, tc, pool, pT_sb, rit):
    """XOR-slot all-gather without tile_critical (those drain every engine
    at exit -- two per body made the v1 path ~85us/body).

    Slot window: g bufs=4; sends of round r are gated on consumed >= r
    (reduce r-1 done locally). Safety chain: peer's write of round r+4
    into my slot requires peer reduce r+2, which requires my send r+2,
    which requires my reduce r+1 > my reduce r -- so slot r%4 is long
    consumed before it is overwritten. In steady state every wait here is
    pre-satisfied (exchange latency ~3us << round period), so no engine
    stalls."""
    g = pool.tile([128, 8, 2 * DC], F32, tag="g", bufs=4)
    nc.vector.tensor_copy(g[:, 0, :], pT_sb[:])
    if rit >= 1:
        nc.gpsimd.wait_ge(nc._consumed, rit)
    for k in range(1, 8):
        rdests = [None] * 8
        rdests[k] = (0, k)
        nc.gpsimd.remote_dma_broadcast(
            g[:, k, :], pT_sb[:],
            remote_sem=nc._rs1, local_sem=nc._ls1,
            rdests=rdests,
        ).then_inc(nc._prep1, 1)
    nc.gpsimd.wait_ge(nc._prep1, 7 * (rit + 1))
    nc.gpsimd.trigger_dma(7)
    hbT = pool.tile([128, 2 * DC], F32, tag="hbT")
    nc.vector.wait_ge(nc._rs1, 14 * (rit + 1))
    nc.vector.tensor_reduce(
        hbT[:], g[:].rearrange("p k f -> p f k"),
        mybir.AxisListType.X, mybir.AluOpType.add,
    ).then_inc(nc._consumed, 1)
    return hbT


def _emit_exchange_remote(nc, tc, pool, pT_sb, rit):
    """hbT = sum over the 8 cores' pT via XOR-slot all-gather + DVE reduce.

    rit is the 0-based REMOTE iteration index (iteration 0 of the program
    always goes through ncfw, whose firmware rendezvous guarantees every
    core is executing -- with per-execution-zeroed semaphores -- before the
    first remote send fires)."""
    g = pool.tile([128, 8, 2 * DC], F32, tag="g", bufs=4)
    nc.vector.tensor_copy(g[:, 0, :], pT_sb[:])
    with tc.tile_critical():
        if rit >= 1:
            # send(rit) >= consume(rit-1) bounds inter-core skew
            nc.gpsimd.wait_ge(nc._consumed, rit)
        for k in range(1, 8):
            rdests = [None] * 8
            rdests[k] = (0, k)
            nc.gpsimd.remote_dma_broadcast(
                g[:, k, :], pT_sb[:],
                remote_sem=nc._rs1, local_sem=nc._ls1,
                rdests=rdests,
            ).then_inc(nc._prep1, 1)
        nc.gpsimd.wait_ge(nc._prep1, 7 * (rit + 1))
        nc.gpsimd.trigger_dma(7)
    hbT = pool.tile([128, 2 * DC], F32, tag="hbT")
    with tc.tile_critical():
        nc.vector.wait_ge(nc._rs1, 14 * (rit + 1))
        nc.vector.tensor_reduce(
            hbT[:], g[:].rearrange("p k f -> p f k"),
            mybir.AxisListType.X, mybir.AluOpType.add,
        ).then_inc(nc._consumed, 1)
    return hbT


def _emit_body_a(nc, tc, pools, const, h_d, big_h):
    """Pipelined-AG stage A: load + partial sums + AllGather doorbell.

    Returns the ctx stage B needs. The gpsimd queue only carries the 8KB
    cc_in store and the PTC doorbell here -- the PTC releases the sequencer
    at doorbell time (collective runs on the TOPSP cores), so consecutive
    bodies' AllGathers overlap once the fetch is deferred to stage B.
    """
    pool, psum1, psum2, dram = pools
    ones2m, m_sb, mask16, ident = const
    it = _ITER[0]
    _ITER[0] += 1

    h_sb = pool.tile([128, 4, D], F32, tag="h", bufs=4)
    if big_h:
        hap = h_d.ap().rearrange("(i p c) d -> i p c d", i=16, p=128)[it % 16]
    else:
        hap = h_d.ap().rearrange("(p c) d -> p c d", p=128)
    nc.sync.dma_start(h_sb[:, 0:1, :], hap[:, 0:1, :])
    nc.scalar.dma_start(h_sb[:, 1:2, :], hap[:, 1:2, :])
    nc.sync.dma_start(h_sb[:, 2:3, :], hap[:, 2:3, :])
    nc.scalar.dma_start(h_sb[:, 3:4, :], hap[:, 3:4, :])

    hsum = pool.tile([128, D], F32, tag="hsum")
    nc.vector.tensor_tensor(hsum[:], h_sb[:, 0, :], h_sb[:, 1, :],
                            mybir.AluOpType.add)
    nc.vector.tensor_tensor(hsum[:], hsum[:], h_sb[:, 2, :],
                            mybir.AluOpType.add)
    nc.vector.tensor_tensor(hsum[:], hsum[:], h_sb[:, 3, :],
                            mybir.AluOpType.add)

    pT_ps = psum1.tile([128, 2 * DC], F32, tag="pT")
    for dc in range(DC):
        nc.tensor.matmul(
            pT_ps[:, dc * 2:dc * 2 + 2],
            hsum[:, dc * 128:(dc + 1) * 128],
            ones2m[:],
            start=True,
            stop=True,
        )
    pT_sb = pool.tile([128, 2 * DC], F32, tag="pTs")
    nc.vector.tensor_copy(pT_sb[:], pT_ps[:])

    pTt_ps = psum1.tile([2 * DC, 128], F32, tag="pTt", bufs=1)
    nc.tensor.matmul(pTt_ps[:], pT_sb[:], ident[:], start=True, stop=True)
    pTt_sb = pool.tile([2 * DC, 128], F32, tag="pTts")
    nc.vector.tensor_copy(pTt_sb[:], pTt_ps[:])

    cc_in = dram.tile([16, 128], F32, tag="cci")
    cc_out = dram.tile([128, 128], F32, tag="cco", addr_space="Shared")
    nc.gpsimd.dma_start(cc_in[:], pTt_sb[:])
    nc.gpsimd.collective_compute(
        "AllGather", mybir.AluOpType.bypass, replica_groups=GROUP,
        ins=[cc_in.opt()], outs=[cc_out.opt()],
    )
    return cc_out


def _emit_body_b(nc, tc, pools, const, out_d, cc_out):
    """Pipelined-AG stage B: fetch gathered partials + compute + store."""
    pool, psum1, psum2, dram = pools
    ones2m, m_sb, mask16, ident = const

    g_sb = pool.tile([128, 128], F32, tag="g")
    nc.gpsimd.dma_start(g_sb[:], cc_out[:])
    hbT_ps = psum1.tile([128, 2 * DC], F32, tag="hbTps", bufs=1)
    nc.tensor.matmul(hbT_ps[:], g_sb[:], mask16[:], start=True, stop=True)
    hbT = pool.tile([128, 2 * DC], F32, tag="hbT")
    nc.vector.tensor_copy(hbT[:], hbT_ps[:])

    selb = pool.tile([128, DC, 2, 64], F32, tag="selb")
    nc.vector.tensor_copy(
        selb[:],
        hbT[:].rearrange("p (dc b) -> p dc b", b=2)
              .unsqueeze(3).broadcast_to([128, DC, 2, 64]),
    )

    bc_ps = psum2.tile([128, D], F32, tag="bc")
    for half in range(2):
        for dc in range(DC):
            nc.tensor.matmul(
                bc_ps[:, half * 512:(half + 1) * 512],
                selb[:, dc, :, :].rearrange("p b r -> p (b r)"),
                m_sb[:, dc, half * 512:(half + 1) * 512],
                start=(dc == 0),
                stop=(dc == DC - 1),
            )
    bc_sb = pool.tile([128, D], bdt, tag="bcs", bufs=3)
    nc.scalar.copy(bc_sb[:], bc_ps[:])

    oap = out_d.ap().rearrange("(p c) d -> p c d", p=128)
    for c in range(4):
        eng = nc.sync if c % 2 == 0 else nc.scalar
        eng.dma_start(oap[:, c:c + 1, :],
                      bc_sb[:].unsqueeze(1).broadcast_to([128, 1, D]))


def _emit_body(nc, tc, pools, const, h_d, out_d, exchange, big_h):
    pool, psum1, psum2, dram = pools
    ones2m, m_sb, mask16, ident = const
    it = _ITER[0]
    bdt = BF16 if exchange in BF_MODES else F32

    if exchange in ("agonly", "agburst"):
        # exchange-chain-only probes: "agonly" = serial store+AG+fetch per
        # body (latency), "agburst" = store+doorbell only, 4-deep window
        # (ncfw AllGather throughput)
        _ITER[0] += 1
        pTt_sb = pool.tile([16, 128], F32, tag="pTts")
        nc.vector.memset(pTt_sb[:], 0.5)
        cc_in = dram.tile([16, 128], F32, tag="cci")
        cc_out = dram.tile([128, 128], F32, tag="cco", addr_space="Shared")
        nc.gpsimd.dma_start(cc_in[:], pTt_sb[:])
        nc.gpsimd.collective_compute(
            "AllGather", mybir.AluOpType.bypass, replica_groups=GROUP,
            ins=[cc_in.opt()], outs=[cc_out.opt()],
        )
        if exchange == "agonly":
            g_sb = pool.tile([128, 128], F32, tag="g")
            nc.gpsimd.dma_start(g_sb[:], cc_out[:])
        return

    # ---- load local hidden slice: partition p = rows 4p..4p+3 (16KB contig),
    # split across both HWDGE rings
    # two half-tiles: the first half's chain (fold + pT matmul + exchange
    # kickoff) starts as soon as chunks 0,1 land, even with whole-tile
    # dependency granularity
    hb = getattr(nc, "_hbufs", 3)
    h_a = pool.tile([128, 2, D], bdt, tag="ha", bufs=hb)
    h_b = pool.tile([128, 2, D], bdt, tag="hb", bufs=hb)
    if big_h:
        # probe-only: body it loads a DISTINCT 2MB slice of a 32MB input so
        # no compiler/HW effect can collapse identical bodies
        hap = h_d.ap().rearrange("(i p c) d -> i p c d", i=16, p=128)[it % 16]
    else:
        hap = h_d.ap().rearrange("(p c) d -> p c d", p=128)
    ds = getattr(nc, "_dsplit", 5)
    if ds == 5:
        # direction-split queues: ALL loads on sync, ALL stores on scalar.
        # Loads have no AllGather dependency, so the load queue prefetches
        # straight through the per-body exchange stall that otherwise
        # blocks loads queued behind AG-dependent stores.
        nc.sync.dma_start(h_a[:], hap[:, 0:2, :])
        nc.sync.dma_start(h_b[:], hap[:, 2:4, :])
    else:
        nc.sync.dma_start(h_a[:, 0:1, :], hap[:, 0:1, :])
        nc.scalar.dma_start(h_a[:, 1:2, :], hap[:, 1:2, :])
        nc.sync.dma_start(h_b[:, 0:1, :], hap[:, 2:3, :])
        nc.scalar.dma_start(h_b[:, 1:2, :], hap[:, 3:4, :])

    # ---- fold the 4 rows per partition on DVE (in place: 3 adds)
    # ---- fold rows in TWO halves so the pT matmuls (and so the exchange)
    # can start as soon as the first two DMA chunks land, ~1-2us earlier
    # than waiting for the full slice; the halves sum in PSUM (f32), which
    # also removes one bf16 rounding step vs a 4-way DVE fold
    hsum = pool.tile([128, 2, D], bdt, tag="hsum")
    nc.vector.tensor_tensor(hsum[:, 0, :], h_a[:, 0, :], h_a[:, 1, :],
                            mybir.AluOpType.add)
    nc.vector.tensor_tensor(hsum[:, 1, :], h_b[:, 0, :], h_b[:, 1, :],
                            mybir.AluOpType.add)

    # ---- per-core partial column sums:
    # pT[p, dc*2+b] = (1/S) * sum_{local rows of batch b} h[row, dc*128+p]
    pT_ps = psum1.tile([128, 2 * DC], F32, tag="pT")
    for dc in range(DC):
        for half in range(2):
            # start/stop pairs must be adjacent per PSUM region
            nc.tensor.matmul(
                pT_ps[:, dc * 2:dc * 2 + 2],
                hsum[:, half, dc * 128:(dc + 1) * 128],
                ones2m[:],
                start=(half == 0),
                stop=(half == 1),
            )
    pT_sb = pool.tile([128, 2 * DC], F32, tag="pTs")
    if exchange == "remote" and it >= 3:
        with tc.tile_critical():
            # remote sends of iteration it-2 (remote index it-3) must have
            # drained before reusing pT_sb's buffer (bufs=2)
            nc.vector.wait_ge(nc._ls1, 112 * (it - 2))
            nc.vector.tensor_copy(pT_sb[:], pT_ps[:])
    elif exchange == "remote2" and it >= 3:
        nc.vector.wait_ge(nc._ls1, 112 * (it - 2))
        nc.vector.tensor_copy(pT_sb[:], pT_ps[:])
    else:
        nc.vector.tensor_copy(pT_sb[:], pT_ps[:])

    # ---- cross-core combine -> hbT
    if exchange in ("ag", "ag2", "agalt", "ags", "agb", "agbx"):
        # transpose via identity matmul: pTt[f, q] = pT[q, f]
        xdt = BF16 if exchange == "agbx" else F32
        pTt_ps = psum1.tile([2 * DC, 128], F32, tag="pTt", bufs=1)
        nc.tensor.matmul(pTt_ps[:], pT_sb[:], ident[:], start=True, stop=True)
        pTt_sb = pool.tile([2 * DC, 128], xdt, tag="pTts")
        nc.vector.tensor_copy(pTt_sb[:], pTt_ps[:])
        cc_eng = None
        if exchange == "agalt":
            # alternate the blocking collective between two queues so two
            # AllGathers can be in flight and each queue only stalls every
            # other body
            cc_eng = nc.tensor if (it % 2) else nc.gpsimd
        hbT = _emit_exchange_ag(nc, tc, pool, psum1, dram, pTt_sb, mask16,
                                hwdge=(exchange == "ag2"), cc_eng=cc_eng,
                                xdt=xdt)
    elif exchange in ("none", "noneb"):
        hbT = pT_sb
    elif exchange == "ncfw" or (
            exchange in ("remote", "remote2", "remote3") and it == 0):
        hbT = _emit_exchange_ncfw(nc, tc, pool, dram, pT_sb)
    elif exchange == "remote3":
        hbT = _emit_exchange_remote3(nc, tc, pool, pT_sb, it - 1)
    elif exchange == "remote2":
        hbT = _emit_exchange_remote2(nc, tc, pool, pT_sb, it - 1)
    else:
        hbT = _emit_exchange_remote(nc, tc, pool, pT_sb, it - 1)
    _ITER[0] += 1
    if exchange == "ags":
        return hbT
    _emit_tail(nc, pools, const, out_d, hbT)


def _emit_tail(nc, pools, const, out_d, hbT):
    pool, psum1, psum2, dram = pools
    ones2m, m_sb, mask16, ident = const
    bdt = m_sb.dtype

    # ---- selb[:, dc, :] as lhsT: lhsT[dq, p] = hbT[dq, dc*2 + p//64]
    selb = pool.tile([128, DC, 2, 64], bdt, tag="selb")
    nc.vector.tensor_copy(
        selb[:],
        hbT[:].rearrange("p (dc b) -> p dc b", b=2)
              .unsqueeze(3).broadcast_to([128, DC, 2, 64]),
    )

    # ---- fused r-compute + partition broadcast:
    # bc[p, dout] = sum_d hbar[b(p), d] * M[d, dout] = r[b(p), dout]
    bc_ps = psum2.tile([128, D], F32, tag="bc")
    for half in range(2):
        for dc in range(DC):
            nc.tensor.matmul(
                bc_ps[:, half * 512:(half + 1) * 512],
                selb[:, dc, :, :].rearrange("p b r -> p (b r)"),
                m_sb[:, dc, half * 512:(half + 1) * 512],
                start=(dc == 0),
                stop=(dc == DC - 1),
            )
    bc_sb = pool.tile([128, D], bdt, tag="bcs", bufs=3)
    nc.scalar.copy(bc_sb[:], bc_ps[:])

    # ---- store: out row 4p+c = bc[p, :]  (b(row) = p//64 for all c),
    # split across both HWDGE rings
    oap = out_d.ap().rearrange("(p c) d -> p c d", p=128)
    ds = getattr(nc, "_dsplit", 5)
    if ds == 5:
        nc.scalar.dma_start(oap[:, 0:2, :],
                            bc_sb[:].unsqueeze(1).broadcast_to([128, 2, D]))
        nc.scalar.dma_start(oap[:, 2:4, :],
                            bc_sb[:].unsqueeze(1).broadcast_to([128, 2, D]))
    elif ds == 2:
        nc.sync.dma_start(oap[:, 0:2, :],
                          bc_sb[:].unsqueeze(1).broadcast_to([128, 2, D]))
        nc.scalar.dma_start(oap[:, 2:4, :],
                            bc_sb[:].unsqueeze(1).broadcast_to([128, 2, D]))
    else:
        for c in range(4):
            eng = nc.sync if c % 2 == 0 else nc.scalar
            if ds == 8:
                eng.dma_start(oap[:, c:c + 1, 0:512],
                              bc_sb[:, 0:512].unsqueeze(1)
                              .broadcast_to([128, 1, 512]))
                eng.dma_start(oap[:, c:c + 1, 512:1024],
                              bc_sb[:, 512:1024].unsqueeze(1)
                              .broadcast_to([128, 1, 512]))
            else:
                eng.dma_start(oap[:, c:c + 1, :],
                              bc_sb[:].unsqueeze(1).broadcast_to([128, 1, D]))


def build(loop_k: int = 0, num_devices: int = N_CORES, compile: bool = True,
          exchange: str | None = None, big_h: bool = False,
          nsends: int | None = None, dsplit: int | None = None,
          xq: int | None = None, hbufs: int | None = None):
    exchange = EXCHANGE if exchange is None else exchange
    nc = bacc.Bacc("TRN2", target_bir_lowering=False, debug=False,
                   num_devices=num_devices)
    if dsplit is not None:
        nc._dsplit = dsplit
    if xq is not None:
        nc._xq = xq
    if hbufs is not None:
        nc._hbufs = hbufs
    _ITER[0] = 0
    if exchange in ("remote", "remote2", "remote3"):
        nc._rs1 = nc.alloc_semaphore("rs1")
        nc._ls1 = nc.alloc_semaphore("ls1")
        nc._prep1 = nc.alloc_semaphore("prep1")
        nc._consumed = nc.alloc_semaphore("consumed")
        if exchange == "remote3":
            nc._rs = [nc.alloc_semaphore(f"rs_w{w}") for w in range(4)]
            if nsends is not None:
                nc._nsends = nsends
    nc.has_collectives = True
    h_rows = 16 * ROWS if big_h else ROWS
    bulk_dt = BF16 if exchange in BF_MODES else F32
    h_d = nc.dram_tensor("h", [h_rows, D], bulk_dt, kind="ExternalInput")
    wv_d = nc.dram_tensor("wv", [D, D], F32, kind="ExternalInput")
    wot_d = nc.dram_tensor("wot", [D, D], F32, kind="ExternalInput")
    aux_d = (nc.dram_tensor("aux", [128, 144], F32, kind="ExternalInput")
             if exchange.startswith("ag") else None)
    out_d = nc.dram_tensor("out", [ROWS, D], bulk_dt, kind="ExternalOutput")

    with tile.TileContext(nc) as tc:
        with (
            tc.tile_pool(name="sbuf", bufs=2) as pool,
            tc.tile_pool(name="psum1", bufs=2, space="PSUM") as psum1,
            tc.tile_pool(name="psum2",
                         bufs=1 if exchange.startswith("ag") else 2,
                         space="PSUM") as psum2,
            tc.tile_pool(name="psumm", bufs=1, space="PSUM") as psum_m0,
            tc.tile_pool(name="dram", bufs=4, space="DRAM") as dram,
        ):
            # ag needs PSUM banks for the transpose/reduce tiles: fold the
            # const-time M-fold PSUM into psum2 so the total stays <= 8 banks
            psum_m = psum2 if exchange.startswith("ag") else psum_m0
            const = _emit_const(nc, tc, pool, psum_m, wv_d, wot_d, exchange,
                                aux_d)
            pools = (pool, psum1, psum2, dram)
            if exchange == "agp":
                depth = 2
                pend = []
                for _ in range(max(1, loop_k)):
                    pend.append(_emit_body_a(nc, tc, pools, const, h_d, big_h))
                    if len(pend) > depth:
                        _emit_body_b(nc, tc, pools, const, out_d, pend.pop(0))
                while pend:
                    _emit_body_b(nc, tc, pools, const, out_d, pend.pop(0))
            elif exchange == "ags":
                # stores one body behind: the sync/scalar queues never stall
                # on the current body's AllGather
                pend = []
                for _ in range(max(1, loop_k)):
                    pend.append(_emit_body(nc, tc, pools, const, h_d, out_d,
                                           exchange, big_h))
                    if len(pend) > 1:
                        _emit_tail(nc, pools, const, out_d, pend.pop(0))
                while pend:
                    _emit_tail(nc, pools, const, out_d, pend.pop(0))
            else:
                for _ in range(max(1, loop_k)):
                    _emit_body(nc, tc, pools, const, h_d, out_d, exchange,
                               big_h)
    if compile:
        nc.compile()
    return nc


def _get(loop_k: int = 0):
    key = (loop_k, EXCHANGE)
    if key not in _BUILT:
        _BUILT[key] = build(loop_k)
    return _BUILT[key]


def _aux_const():
    aux = np.zeros((128, 144), np.float32)
    for p in range(128):
        aux[p, p % 16] = 1.0          # mask16
        aux[p, 16 + p] = 1.0          # identity
    return aux


def make_in_maps(hidden_states, Wv, Wo, mode=None):
    mode = EXCHANGE if mode is None else mode
    hidden_states = np.asarray(hidden_states, dtype=np.float32)
    Wv = np.ascontiguousarray(np.asarray(Wv, dtype=np.float32))
    WoT = np.ascontiguousarray(np.asarray(Wo, dtype=np.float32).T)
    aux = _aux_const()
    if mode in BF_MODES:
        import ml_dtypes
        hidden_states = hidden_states.astype(ml_dtypes.bfloat16)
    in_maps = []
    for c in range(N_CORES):
        sl = slice(c * S_LOC, (c + 1) * S_LOC)
        in_maps.append({
            "h": np.ascontiguousarray(hidden_states[:, sl, :]).reshape(ROWS, D),
            "wv": Wv,
            "wot": WoT,
            "aux": aux,
        })
    return in_maps


def assemble(results):
    out = np.empty((B, S, D), np.float32)
    for c in range(N_CORES):
        o = results[c]["out"].reshape(B, S_LOC, D)
        if o.dtype != np.float32:
            o = o.astype(np.float32)
        out[:, c * S_LOC:(c + 1) * S_LOC, :] = o
    return out


def kernel(hidden_states, Wq=None, Wk=None, Wv=None, Wo=None, **_unused):
    nc = _get(0)
    in_maps = make_in_maps(hidden_states, Wv, Wo)
    res = run_bass_kernel_spmd(nc, in_maps, list(range(N_CORES)))
    return assemble(res.results)


if __name__ == "__main__":
    rng = np.random.default_rng(0)
    h = rng.standard_normal((B, S, D), dtype=np.float32)
    wv = rng.standard_normal((D, D), dtype=np.float32) * 0.02
    wo = rng.standard_normal((D, D), dtype=np.float32) * 0.02
    out = kernel(h, None, None, wv, wo)
    ref = (h.mean(axis=1) @ wv.T @ wo.T)[:, None, :] * np.ones((1, S, 1), np.float32)
    err = np.abs(out - ref).max() / np.abs(ref).max()
    print("self-check rel err:", err)
